# revision 22
# baseline (speedup 1.0000x reference)
"""Distributed transformer-block kernel for one TRN2 chip (8 NeuronCores).

Reference computation (S=4096, N=1024):
    xn = LayerNorm(x) * g + b
    q,k,v = xn@Wq+bq, xn@Wk+bk, xn@Wv+bv
    w = softmax((k @ q.T) / sqrt(N), axis=-1)
    h = w @ v
    out = leaky_relu(h@W1+b1, 0.1) @ W2 + b2 + xn

Sharding: sequence-parallel. Each core owns 512 rows of x, computes its
q/k/v shard, all-gathers xn and v, then computes its 512-row slice of
attention + FFN fully locally.

Fast path (zero biases, unit LN affine — the shipped problem): all five
matmul groups run in fp8(e4m3) DoubleRow mode (K=256 per instruction,
2x PE throughput). Weights are host-packed into row-pair layout and
pre-scaled by 32/64 so fp8 sees ~unit-sigma values; every rescale is an
exact power of 2, compensated in the exp scale/bias and the final
1/sumexp epilogue (leaky_relu is positively homogeneous):
    host: wk,wq,wv,w1 x32; w2 x64
    kT/kappa/v evac x2^-5  -> true-scale fp8
    exp bias -ln4          -> wT' = exp(logit)/4 (max ~67 < 240 e4m3 max)
    hT evac x2^-6          -> hT' = hT_un * 2^-8
    ff1 evac x1/4 -> 2^-5 ff1_un ; ff2 psum = 2 ff2_un
    sum_row = 8*sume' = 2*sume ;  out = po/(8*sume') + xn

Collectives are fp8 and split in halves (xnT by column-half, v by
row-pair) so transfers pipeline with the projections and are consumed
in arrival order (logits sub 0/1 then 2/3; h even pairs then odd).

Logits computed TRANSPOSED with Wq folded into the k side:
wT[j,i] = xn_full[j] . (Wq @ k_loc[i]), so the gather operand (xn) is
ready right after LayerNorm and feeds the collective early.
"""

import sys

sys.path.insert(0, "/opt/trn_rl_repo")

import numpy as np
import ml_dtypes

import concourse.bass as bass
from concourse import bacc, tile, mybir
from concourse.bass import ts
from concourse.bass_utils import run_bass_kernel_spmd
from concourse.masks import make_identity

F32 = mybir.dt.float32
BF16 = mybir.dt.bfloat16
F8 = mybir.dt.float8e4
AF = mybir.ActivationFunctionType
DR = mybir.MatmulPerfMode.DoubleRow

P = 128
R = 8            # cores
S = 4096         # sequence
N = 1024         # hidden
FF = 4096        # ffn hidden
SL = S // R      # local rows (512)
HSL = SL // 2    # 256
NK = N // P      # 8 hidden chunks
NKH = NK // 2    # 4 hidden pair-chunks
NI = SL // P     # 4 local row chunks
NJ = S // P      # 32 global row chunks
NF = FF // P     # 32 ffn chunks
NFH = NF // 2    # 16 ffn pair-chunks
SCALE = 1.0 / np.sqrt(N).astype(np.float32)  # 0.03125
EPS = 1e-5
NEG_LN4 = float(-np.log(4.0))

_cached = None
_DBG = False


def _build_fp8():
    nc = bacc.Bacc("TRN2", target_bir_lowering=False, debug=False, num_devices=R)

    x_e = nc.declare_dram_parameter("x", [SL, N], F32, isOutput=False)
    wkp_e = nc.declare_dram_parameter("wkp", [N // 2, 2 * N], F8, isOutput=False)
    wqp_e = nc.declare_dram_parameter("wqp", [N // 2, 2 * N], F8, isOutput=False)
    wvp_e = nc.declare_dram_parameter("wvp", [N // 2, 2 * N], F8, isOutput=False)
    w1p_e = nc.declare_dram_parameter("w1p", [N // 2, 2 * FF], F8, isOutput=False)
    w2p_e = nc.declare_dram_parameter("w2p", [FF // 2, 2 * N], F8, isOutput=False)
    out_e = nc.declare_dram_parameter("out", [SL, N], F32, isOutput=True)
    if _DBG:
        d_vt = nc.declare_dram_parameter("d_vt", [2 * R, P, 2 * N], F32, isOutput=True)
        d_acc = nc.declare_dram_parameter("d_acc", [P, SL], F32, isOutput=True)
        d_hT = nc.declare_dram_parameter("d_hT", [NK, P, SL], F32, isOutput=True)
        d_ff1 = nc.declare_dram_parameter("d_ff1", [4, P, SL], F32, isOutput=True)
        d_rec = nc.declare_dram_parameter("d_rec", [P, NI], F32, isOutput=True)
        d_wTs = nc.declare_dram_parameter("d_wTs", [4, P, SL], F32, isOutput=True)

    # collective bounce buffers (fp8), split in halves for pipelining
    agq_in = nc.dram_tensor("agq_in", [NK, P, SL], F8)
    agq_out = nc.dram_tensor("agq_out", [R * NK, P, SL], F8, addr_space="Shared")
    agv_in_a = nc.dram_tensor("agv_in_a", [2, P, N], F8)
    agv_in_b = nc.dram_tensor("agv_in_b", [2, P, N], F8)
    agv_out_a = nc.dram_tensor("agv_out_a", [R * 2, P, N], F8, addr_space="Shared")
    agv_out_b = nc.dram_tensor("agv_out_b", [R * 2, P, N], F8, addr_space="Shared")
    dum_in = nc.dram_tensor("dum_in", [1, P, 1], F32)
    dum_out = nc.dram_tensor("dum_out", [R, P, 1], F32, addr_space="Shared")

    rg = [list(range(R))]

    def enter(cm):
        return cm, cm.__enter__()

    def leave(cm):
        cm.__exit__(None, None, None)

    with tile.TileContext(nc) as tc:
        base_cm, basep = enter(tc.tile_pool(name="base", bufs=1))

        dum_t = basep.tile([P, 1], F32)
        with tc.high_priority():
            nc.gpsimd.memset(dum_t[:], 0.0)
            nc.gpsimd.dma_start(dum_in[0], dum_t[:])
            nc.gpsimd.collective_compute(
                "AllGather", mybir.AluOpType.bypass, replica_groups=rg,
                ins=[dum_in[:]], outs=[dum_out[:]],
            )
        ident = basep.tile([P, P], BF16)
        make_identity(nc, ident)
        ones_col_f = basep.tile([P, 1], F32)
        nc.gpsimd.memset(ones_col_f[:], 1.0)
        one_f = basep.tile([1, 1], F32)
        nc.gpsimd.memset(one_f[:], 1.0)
        zero_col = basep.tile([P, 1], F32)
        nc.gpsimd.memset(zero_col[:], 0.0)
        eps_col = basep.tile([P, 1], F32)
        nc.gpsimd.memset(eps_col[:], EPS)
        nln4_col = basep.tile([P, 1], F32)
        nc.gpsimd.memset(nln4_col[:], NEG_LN4)

        xn_sb = basep.tile([P, NI * N], BF16)    # normed x, natural (residual)
        sum_row_f = basep.tile([1, SL], F32)
        recip_col = basep.tile([P, NI], F32)

        # x loads first (LayerNorm is the critical path at startup), then
        # the resident fp8 weights behind them on the same sync queue
        wts_cm, wts = enter(tc.tile_pool(name="wts", bufs=1, side="left"))
        wkp = [wts.tile([P, 2, N], F8, tag=f"wkp{i}", name=f"wkp{i}") for i in range(NKH)]
        wqp = [wts.tile([P, 2, N], F8, tag=f"wqp{i}", name=f"wqp{i}") for i in range(NKH)]
        wvp = [wts.tile([P, 2, N], F8, tag=f"wvp{i}", name=f"wvp{i}") for i in range(NKH)]
        w1_cm, w1pool = enter(tc.tile_pool(name="w1pool", bufs=1, side="left"))
        w1p = [w1pool.tile([P, 2, FF], F8, tag=f"w1p{i}", name=f"w1p{i}") for i in range(NKH)]
        xs_cm, xs = enter(tc.tile_pool(name="xs", bufs=1))
        xts = [xs.tile([P, N], F32, tag=f"xt{i}", name=f"xt{i}") for i in range(NI)]
        for i in range(NI):
            (nc.sync if i % 2 == 0 else nc.scalar).dma_start(xts[i][:], x_e[ts(i, P), :])
        for i in range(NKH):
            nc.sync.dma_start(wkp[i][:], wkp_e[ts(i, P), :].rearrange("p (two n) -> p two n", two=2))
        for i in range(NKH):
            nc.sync.dma_start(wqp[i][:], wqp_e[ts(i, P), :].rearrange("p (two n) -> p two n", two=2))
        for i in range(NKH):
            nc.sync.dma_start(wvp[i][:], wvp_e[ts(i, P), :].rearrange("p (two n) -> p two n", two=2))
        for i in range(NKH):
            nc.sync.dma_start(w1p[i][:], w1p_e[ts(i, P), :].rearrange("p (two n) -> p two n", two=2))

        # =========== Phase 0: layernorm + transpose ===========
        xnT_cm, xnTp = enter(tc.tile_pool(name="xnTp", bufs=1, side="left"))
        xnT_sb = xnTp.tile([P, NK, SL], F8)

        with (
            tc.tile_pool(name="ln", bufs=4) as ln,
            tc.tile_pool(name="lnsq", bufs=2) as lnsq,
            tc.tile_pool(name="tpsum", bufs=8, space="PSUM") as tpsum,
        ):
            for i in range(NI):
                xt = xts[i]
                sum_t = ln.tile([P, 1], F32, tag="sum")
                nc.vector.reduce_sum(sum_t[:], xt[:], axis=mybir.AxisListType.X)
                sq_scr = lnsq.tile([P, N], BF16, tag="sq")
                sumsq_t = ln.tile([P, 1], F32, tag="sumsq")
                nc.scalar.activation(sq_scr[:], xt[:], AF.Square, bias=zero_col[:], accum_out=sumsq_t[:])
                mu_t = ln.tile([P, 1], F32, tag="mu")
                nc.gpsimd.tensor_scalar_mul(mu_t[:], sum_t[:], 1.0 / N)
                var_t = ln.tile([P, 1], F32, tag="var")
                nc.gpsimd.tensor_scalar_mul(var_t[:], sumsq_t[:], 1.0 / N)
                musq_t = ln.tile([P, 1], F32, tag="musq")
                nc.gpsimd.tensor_mul(musq_t[:], mu_t[:], mu_t[:])
                nc.gpsimd.tensor_sub(var_t[:], var_t[:], musq_t[:])
                std_t = ln.tile([P, 1], F32, tag="std")
                nc.scalar.activation(std_t[:], var_t[:], AF.Sqrt, bias=eps_col[:])
                rstd_t = ln.tile([P, 1], F32, tag="rstd")
                nc.vector.reciprocal(rstd_t[:], std_t[:])
                nmr_t = ln.tile([P, 1], F32, tag="nmr")
                nc.gpsimd.tensor_mul(nmr_t[:], mu_t[:], rstd_t[:])
                nc.gpsimd.tensor_scalar_mul(nmr_t[:], nmr_t[:], -1.0)
                # xn_sb = (x-mu)*rstd : with g=1,b=0 this is final
                xn_i = xn_sb[:, ts(i, N)]
                nc.scalar.activation(xn_i, xt[:], AF.Identity, scale=rstd_t[:], bias=nmr_t[:])
                for k in range(NK):
                    pt = tpsum.tile([P, P], BF16, tag="pt")
                    nc.tensor.transpose(pt[:], xn_sb[:, i * N + k * P : i * N + (k + 1) * P], ident[:])
                    nc.vector.tensor_copy(xnT_sb[:, k, ts(i, P)], pt[:])

        # =========== Phase 1: gathers + projections ===========
        # one xnT gather: logits consume it only after the local projections,
        # so splitting it buys nothing but extra mesh overhead
        nc.gpsimd.dma_start(agq_in[:].rearrange("k p s -> p k s"), xnT_sb[:, :, :])
        nc.gpsimd.collective_compute(
            "AllGather", mybir.AluOpType.bypass, replica_groups=rg,
            ins=[agq_in[:]], outs=[agq_out[:]],
        )

        kv_cm, kvp = enter(tc.tile_pool(name="kvp", bufs=1, side="right"))
        kT_sb = kvp.tile([P, NK, SL], F8)
        rhs_sb = kvp.tile([P, NK, SL], F8)   # kappa = Wq @ k, fp8
        v_sb = kvp.tile([P, NI, N], F8)

        with tc.tile_pool(name="qpsum", bufs=6, space="PSUM") as qpsum:
            # kT[m, i] = sum_n Wk[n, m] xnT[n, i]
            for m in range(NK):
                pk = qpsum.tile([P, SL], F32, tag="pq")
                for pi in range(NKH):
                    nc.tensor.matmul(
                        pk[:],
                        wkp[pi][:, :, ts(m, P)],
                        xnT_sb[:, 2 * pi : 2 * pi + 2, :],
                        start=(pi == 0), stop=(pi == NKH - 1), perf_mode=DR,
                    )
                nc.vector.tensor_scalar_mul(kT_sb[:, m, :], pk[:], 2.0 ** -5)
            # kappa[m, i] = sum_n Wq[m, n] kT[n, i]
            for m in range(NK):
                pq = qpsum.tile([P, SL], F32, tag="pq")
                for pi in range(NKH):
                    nc.tensor.matmul(
                        pq[:],
                        wqp[pi][:, :, ts(m, P)],
                        kT_sb[:, 2 * pi : 2 * pi + 2, :],
                        start=(pi == 0), stop=(pi == NKH - 1), perf_mode=DR,
                    )
                nc.vector.tensor_scalar_mul(rhs_sb[:, m, :], pq[:], 2.0 ** -5)
            # v[i, c] = sum_n xnT[n, i] Wv[n, c]; cb-inner shares the
            # stationary xnT block so the second matmul skips ldweights
            for i in range(NI):
                pv = [qpsum.tile([P, 512], F32, tag="pq", name=f"pv{i}_{cb}") for cb in range(2)]
                for pi in range(NKH):
                    for cb in range(2):
                        mmi = nc.tensor.matmul(
                            pv[cb][:],
                            xnT_sb[:, 2 * pi : 2 * pi + 2, ts(i, P)],
                            wvp[pi][:, :, ts(cb, 512)],
                            start=(pi == 0), stop=(pi == NKH - 1), perf_mode=DR,
                        )
                        if cb == 1:
                            mmi.ins.ldweights = False
                for cb in range(2):
                    nc.vector.tensor_scalar_mul(v_sb[:, i, ts(cb, 512)], pv[cb][:], 2.0 ** -5)
                # launch v-gather halves as soon as their chunks are done
                if i == 1:
                    nc.gpsimd.dma_start(agv_in_a[:].rearrange("k p n -> p k n"), v_sb[:, 0:2, :])
                    nc.gpsimd.collective_compute(
                        "AllGather", mybir.AluOpType.bypass, replica_groups=rg,
                        ins=[agv_in_a[:]], outs=[agv_out_a[:]],
                    )
                if i == 3:
                    nc.gpsimd.dma_start(agv_in_b[:].rearrange("k p n -> p k n"), v_sb[:, 2:4, :])
                    nc.gpsimd.collective_compute(
                        "AllGather", mybir.AluOpType.bypass, replica_groups=rg,
                        ins=[agv_in_b[:]], outs=[agv_out_b[:]],
                    )
        leave(xnT_cm)
        leave(xs_cm)

        # =========== Phase 2: logits (transposed) + exp + running sum ===========
        wT_cm, wTp = enter(tc.tile_pool(name="wTp", bufs=1, side="left"))
        wT_sb = wTp.tile([P, NJ, SL], F8)
        acc = wTp.tile([P, SL], F32)
        nc.vector.memset(acc[:], 0.0)
        # whole gathered v kept resident (4MB fp8); loads overlap phase 2
        vt_cm, vtp = enter(tc.tile_pool(name="vtp", bufs=1, side="left"))
        vt = [vtp.tile([P, 2, N], F8, tag=f"vt{b}", name=f"vt{b}") for b in range(2 * R)]
        with (
            tc.tile_pool(name="qf", bufs=3) as qfp,
            tc.tile_pool(name="wpsum", bufs=3, space="PSUM") as wpsum,
        ):
            def logits_chunk(qf, jc, sub_in_half):
                pw = wpsum.tile([P, SL], F32, tag="pw")
                for pi in range(NKH):
                    nc.tensor.matmul(
                        pw[:],
                        qf[:, 2 * pi : 2 * pi + 2, ts(sub_in_half, P)],
                        rhs_sb[:, 2 * pi : 2 * pi + 2, :],
                        start=(pi == 0), stop=(pi == NKH - 1), perf_mode=DR,
                    )
                nc.scalar.activation(
                    wT_sb[:, jc, :], pw[:], AF.Exp, scale=float(SCALE), bias=nln4_col[:]
                )
                nc.vector.tensor_add(acc[:], acc[:], wT_sb[:, jc, :])

            for rank in range(R):
                qf = qfp.tile([P, NK, SL], F8, tag="qf")
                nc.sync.dma_start(qf[:], agq_out[rank * NK : (rank + 1) * NK].rearrange("k p s -> p k s"))
                for sub in range(NI):
                    logits_chunk(qf, rank * NI + sub, sub)
                nc.gpsimd.dma_start(vt[2 * rank][:], agv_out_a[2 * rank : 2 * rank + 2].rearrange("k p n -> p k n"))
            # odd vt tiles gate on the second v-gather; keep them BEHIND all
            # even loads so they don't head-of-line block the h phase
            for rank in range(R):
                nc.gpsimd.dma_start(vt[2 * rank + 1][:], agv_out_b[2 * rank : 2 * rank + 2].rearrange("k p n -> p k n"))
        leave(kv_cm)

        # =========== Phase 3: hT accumulation ===========
        mid_cm, midp = enter(tc.tile_pool(name="midp", bufs=1, side="right"))
        hT_sb = midp.tile([P, NK, SL], F8)
        ff1T_sb = midp.tile([P, NF, SL], F8)
        # w2 resident; loads overlap attention
        w2_cm, w2pool = enter(tc.tile_pool(name="w2pool", bufs=1, side="right"))
        w2p = [w2pool.tile([P, 2, N], F8, tag=f"w2p{t}", name=f"w2p{t}") for t in range(NFH)]
        for t in range(NFH):
            nc.sync.dma_start(
                w2p[t][:], w2p_e[ts(t, P), :].rearrange("p (two n) -> p two n", two=2)
            )
        with tc.tile_pool(name="hpsum", bufs=1, space="PSUM") as hpsum:
            ph = [hpsum.tile([P, SL], F32, tag=f"ph{c}", name=f"ph{c}") for c in range(NK)]
            border = [2 * r for r in range(R)] + [2 * r + 1 for r in range(R)]
            for step, b in enumerate(border):
                for c in range(NK):
                    nc.tensor.matmul(
                        ph[c][:],
                        vt[b][:, :, ts(c, P)],
                        wT_sb[:, 2 * b : 2 * b + 2, :],
                        start=(step == 0), stop=(step == 2 * R - 1), perf_mode=DR,
                    )
            for c in range(NK):
                if c % 2 == 0:
                    nc.scalar.activation(hT_sb[:, c, :], ph[c][:], AF.Identity, scale=2.0 ** -6, bias=zero_col[:])
                else:
                    nc.vector.tensor_scalar_mul(hT_sb[:, c, :], ph[c][:], 2.0 ** -6)
        if _DBG:
            with tc.tile_pool(name="dbgp", bufs=2) as dbgp:
                for b in range(2 * R):
                    vtf = dbgp.tile([P, 2 * N], F32, tag="dbgv")
                    nc.vector.tensor_copy(vtf[:], vt[b][:].rearrange("p two n -> p (two n)"))
                    nc.sync.dma_start(d_vt[b], vtf[:])
                nc.sync.dma_start(d_acc[:], acc[:])
                for c in range(NK):
                    htf = dbgp.tile([P, SL], F32, tag="dbgs")
                    nc.vector.tensor_copy(htf[:], hT_sb[:, c, :])
                    nc.sync.dma_start(d_hT[c], htf[:])
                for jc in range(4):
                    wtf = dbgp.tile([P, SL], F32, tag="dbgs")
                    nc.vector.tensor_copy(wtf[:], wT_sb[:, 16 + jc, :])
                    nc.sync.dma_start(d_wTs[jc], wtf[:])
        # sumexp finalize: sum over partitions, then 1/(32*sume')
        with tc.tile_pool(name="spsum", bufs=2, space="PSUM") as spsum:
            ps = spsum.tile([1, SL], F32, tag="ps")
            nc.tensor.matmul(ps[:], ones_col_f[:], acc[:])
            nc.vector.tensor_scalar_mul(sum_row_f[:1, :], ps[:1, :], 8.0)
            for ic in range(NI):
                pr = spsum.tile([P, 1], F32, tag="pr")
                nc.tensor.matmul(pr[:], sum_row_f[:1, ts(ic, P)], one_f[:1, :])
                nc.vector.reciprocal(recip_col[:, ic : ic + 1], pr[:])
        leave(vt_cm)
        leave(wT_cm)

        # =========== Phase 4: FFN1 ===========
        with tc.tile_pool(name="fpsum", bufs=3, space="PSUM") as fpsum:
            for f in range(NF):
                pf = fpsum.tile([P, SL], F32, tag="pf")
                for pi in range(NKH):
                    nc.tensor.matmul(
                        pf[:],
                        w1p[pi][:, :, ts(f, P)],
                        hT_sb[:, 2 * pi : 2 * pi + 2, :],
                        start=(pi == 0), stop=(pi == NKH - 1), perf_mode=DR,
                    )
                nc.scalar.activation(ff1T_sb[:, f, :], pf[:], AF.Lrelu, alpha=0.1, scale=0.25, bias=zero_col[:])
        leave(w1_cm)
        leave(wts_cm)
        if _DBG:
            with tc.tile_pool(name="dbgp2", bufs=2) as dbgp2:
                for f in range(4):
                    f1f = dbgp2.tile([P, SL], F32, tag="dbgs2")
                    nc.vector.tensor_copy(f1f[:], ff1T_sb[:, f, :])
                    nc.sync.dma_start(d_ff1[f], f1f[:])
                nc.sync.dma_start(d_rec[:], recip_col[:])

        # =========== Phase 5: FFN2 + epilogue ===========
        with (
            tc.tile_pool(name="outp", bufs=8) as outp,
            tc.tile_pool(name="opsum", bufs=1, space="PSUM") as opsum,
        ):
            po = [opsum.tile([P, 512], F32, tag=f"po{g}", name=f"po{g}") for g in range(NI * 2)]
            # bank g skips step 8+g in the main sweep; appended at the end
            # (stop staggering) so epilogues overlap the final matmuls
            for t in range(NFH):
                prev_loaded = None
                for g in range(NI * 2):
                    if t == NFH - 8 + g:
                        continue
                    ic, mb = g // 2, g % 2
                    mmi = nc.tensor.matmul(
                        po[g][:],
                        ff1T_sb[:, 2 * t : 2 * t + 2, ts(ic, P)],
                        w2p[t][:, :, ts(mb, 512)],
                        start=(t == 0), stop=False, perf_mode=DR,
                    )
                    if prev_loaded == (t, ic):
                        mmi.ins.ldweights = False
                    prev_loaded = (t, ic)
            for g in range(NI * 2):
                ic, mb = g // 2, g % 2
                t = NFH - 8 + g
                nc.tensor.matmul(
                    po[g][:],
                    ff1T_sb[:, 2 * t : 2 * t + 2, ts(ic, P)],
                    w2p[t][:, :, ts(mb, 512)],
                    start=False, stop=True, perf_mode=DR,
                )
                ot = outp.tile([P, 512], F32, tag="ot")
                if g % 2 == 0:
                    nc.vector.scalar_tensor_tensor(
                        ot[:], po[g][:], recip_col[:, ic : ic + 1],
                        xn_sb[:, ic * N + mb * 512 : ic * N + (mb + 1) * 512],
                        op0=mybir.AluOpType.mult, op1=mybir.AluOpType.add,
                    )
                else:
                    nc.scalar.activation(ot[:], po[g][:], AF.Identity, scale=recip_col[:, ic : ic + 1])
                    nc.vector.tensor_add(
                        ot[:], ot[:], xn_sb[:, ic * N + mb * 512 : ic * N + (mb + 1) * 512]
                    )
                oeng = (nc.sync, nc.scalar, nc.gpsimd)[g % 3]
                oeng.dma_start(out_e[ts(ic, P), ts(mb, 512)], ot[:])
        leave(w2_cm)
        leave(mid_cm)
        leave(base_cm)

    nc.compile()
    return nc


def _packpair(W):
    """[K, M] -> [K//2, 2M]: row-pair layout for DoubleRow lhsT/rhs tiles."""
    K, M = W.shape
    return np.ascontiguousarray(
        W.reshape(K // 256, 2, 128, M).transpose(0, 2, 1, 3).reshape(K // 2, 2 * M)
    )


def _fp8_in_maps(inputs):
    p8 = lambda a: np.asarray(a, dtype=np.float32).astype(ml_dtypes.float8_e4m3)
    f = lambda a: np.ascontiguousarray(np.asarray(a, dtype=np.float32))
    Wq = np.asarray(inputs["Wq"], dtype=np.float32)
    Wk = np.asarray(inputs["Wk"], dtype=np.float32)
    Wv = np.asarray(inputs["Wv"], dtype=np.float32)
    W1 = np.asarray(inputs["W1"], dtype=np.float32)
    W2 = np.asarray(inputs["W2"], dtype=np.float32)
    common = {
        "wkp": p8(_packpair(Wk * 32.0)),
        "wqp": p8(_packpair(np.ascontiguousarray(Wq.T) * 32.0)),
        "wvp": p8(_packpair(Wv * 32.0)),
        "w1p": p8(_packpair(W1 * 32.0)),
        "w2p": p8(_packpair(W2 * 64.0)),
    }
    x = f(inputs["x"])
    return [dict(common, x=np.ascontiguousarray(x[r * SL : (r + 1) * SL])) for r in range(R)]


# ---------------------------------------------------------------------------
# General (bf16) fallback for nonzero biases / non-unit LN affine.
# ---------------------------------------------------------------------------

def _build(zero_bias):
    nc = bacc.Bacc("TRN2", target_bir_lowering=False, debug=False, num_devices=R)

    x_e = nc.declare_dram_parameter("x", [SL, N], F32, isOutput=False)
    g_e = nc.declare_dram_parameter("norm_g", [N], F32, isOutput=False)
    bn_e = nc.declare_dram_parameter("norm_b", [N], F32, isOutput=False)
    wq_e = nc.declare_dram_parameter("wq", [N, N], BF16, isOutput=False)
    bq_e = nc.declare_dram_parameter("bq", [N], F32, isOutput=False)
    wk_e = nc.declare_dram_parameter("wk", [N, N], BF16, isOutput=False)
    bk_e = nc.declare_dram_parameter("bk", [N], F32, isOutput=False)
    wv_e = nc.declare_dram_parameter("wv", [N, N], BF16, isOutput=False)
    bv_e = nc.declare_dram_parameter("bv", [N], BF16, isOutput=False)
    w1_e = nc.declare_dram_parameter("w1", [N, FF], BF16, isOutput=False)
    b1_e = nc.declare_dram_parameter("b1", [FF], BF16, isOutput=False)
    w2_e = nc.declare_dram_parameter("w2", [FF, N], BF16, isOutput=False)
    b2_e = nc.declare_dram_parameter("b2", [N], BF16, isOutput=False)
    out_e = nc.declare_dram_parameter("out", [SL, N], F32, isOutput=True)

    # collective bounce buffers
    agq_in = nc.dram_tensor("agq_in", [NK, P, SL], BF16)
    agq_out = nc.dram_tensor("agq_out", [R * NK, P, SL], BF16, addr_space="Shared")
    agv_in = nc.dram_tensor("agv_in", [NI, P, N], BF16)
    agv_out = nc.dram_tensor("agv_out", [R * NI, P, N], BF16, addr_space="Shared")

    rg = [list(range(R))]

    def enter(cm):
        return cm, cm.__enter__()

    def leave(cm):
        cm.__exit__(None, None, None)

    with tile.TileContext(nc) as tc:
        base_cm, base = enter(tc.tile_pool(name="base", bufs=1))

        # ---- whole-kernel constants / carriers ----
        ident = base.tile([P, P], BF16)
        make_identity(nc, ident)
        ones_row_b = base.tile([1, P], BF16)
        nc.gpsimd.memset(ones_row_b[:], 1.0)
        ones_col_f = base.tile([P, 1], F32)
        nc.gpsimd.memset(ones_col_f[:], 1.0)
        one_f = base.tile([1, 1], F32)
        nc.gpsimd.memset(one_f[:], 1.0)
        zero_col = base.tile([P, 1], F32)
        nc.gpsimd.memset(zero_col[:], 0.0)
        eps_col = base.tile([P, 1], F32)
        nc.gpsimd.memset(eps_col[:], EPS)

        xn_sb = base.tile([P, NI * N], BF16)    # normed x, natural layout (residual)
        sum_row_f = base.tile([1, SL], F32)
        sum_row_b = base.tile([1, SL], BF16)
        recip_col = base.tile([P, NI], F32)

        # =========== Phase 0: layernorm + transpose ===========
        xnT_cm, xnTp = enter(tc.tile_pool(name="xnTp", bufs=1, side="left"))
        xnT_sb = xnTp.tile([P, NK * SL], BF16)

        # per-partition views of the LN affine for the transposed layout
        g_col = base.tile([P, NK], F32)
        nc.sync.dma_start(g_col[:], g_e[:].rearrange("(m p) -> p m", p=P))
        b_col = base.tile([P, NK], F32)
        nc.sync.dma_start(b_col[:], bn_e[:].rearrange("(m p) -> p m", p=P))

        with (
            tc.tile_pool(name="xs", bufs=4) as xs,
            tc.tile_pool(name="ln", bufs=4) as ln,
            tc.tile_pool(name="tpsum", bufs=8, space="PSUM") as tpsum,
        ):
            for i in range(NI):
                xt = xs.tile([P, N], F32, tag="xt")
                nc.sync.dma_start(xt[:], x_e[ts(i, P), :])
                sum_t = ln.tile([P, 1], F32, tag="sum")
                nc.vector.reduce_sum(sum_t[:], xt[:], axis=mybir.AxisListType.X)
                sq_scr = lnsq.tile([P, N], BF16, tag="sq")
                sumsq_t = ln.tile([P, 1], F32, tag="sumsq")
                nc.scalar.activation(sq_scr[:], xt[:], AF.Square, bias=zero_col[:], accum_out=sumsq_t[:])
                mu_t = ln.tile([P, 1], F32, tag="mu")
                nc.gpsimd.tensor_scalar_mul(mu_t[:], sum_t[:], 1.0 / N)
                var_t = ln.tile([P, 1], F32, tag="var")
                nc.gpsimd.tensor_scalar_mul(var_t[:], sumsq_t[:], 1.0 / N)
                musq_t = ln.tile([P, 1], F32, tag="musq")
                nc.gpsimd.tensor_mul(musq_t[:], mu_t[:], mu_t[:])
                nc.gpsimd.tensor_sub(var_t[:], var_t[:], musq_t[:])
                std_t = ln.tile([P, 1], F32, tag="std")
                nc.scalar.activation(std_t[:], var_t[:], AF.Sqrt, bias=eps_col[:])
                rstd_t = ln.tile([P, 1], F32, tag="rstd")
                nc.vector.reciprocal(rstd_t[:], std_t[:])
                nmr_t = ln.tile([P, 1], F32, tag="nmr")
                nc.gpsimd.tensor_mul(nmr_t[:], mu_t[:], rstd_t[:])
                nc.gpsimd.tensor_scalar_mul(nmr_t[:], nmr_t[:], -1.0)
                # xn_sb holds z = (x-mu)*rstd (bf16); affine for the residual
                # is applied in-place later, off the critical path
                xn_i = xn_sb[:, ts(i, N)]
                nc.scalar.activation(xn_i, xt[:], AF.Identity, scale=rstd_t[:], bias=nmr_t[:])
                for k in range(NK):
                    pt = tpsum.tile([P, P], BF16, tag="pt")
                    nc.tensor.transpose(pt[:], xn_sb[:, i * N + k * P : i * N + (k + 1) * P], ident[:])
                    # affine fused here: in transposed layout g,b are per-partition
                    nc.scalar.activation(
                        xnT_sb[:, k * SL + i * P : k * SL + (i + 1) * P], pt[:], AF.Identity,
                        scale=g_col[:, k : k + 1], bias=b_col[:, k : k + 1],
                    )

        # =========== Phase 1: projections + all-gathers ===========
        # zero_bias path: gather xnT itself (ready far earlier than q), and
        # fold Wq into the k side:  logits = xnT_full . (Wq @ kT)  — same
        # matmul count, but the collective launches ~35us sooner.
        kT_cm, kTp = enter(tc.tile_pool(name="kTp", bufs=1, side="right"))
        kT_sb = kTp.tile([P, NK * SL], BF16)
        rhs_sb = kTp.tile([P, NK * SL], BF16)  # logits rhs: kappa^T (zero_bias) or kT

        if zero_bias:
            for m in range(NK):
                (nc.gpsimd if m % 2 == 0 else nc.scalar).dma_start(agq_in[m], xnT_sb[:, ts(m, SL)])
            nc.gpsimd.collective_compute(
                "AllGather", mybir.AluOpType.bypass, replica_groups=rg,
                ins=[agq_in[:]], outs=[agq_out[:]],
            )

        qkv_cm, qkv = enter(tc.tile_pool(name="qkv", bufs=1, side="right"))
        bq_col = qkv.tile([P, NK], F32)
        nc.sync.dma_start(bq_col[:], bq_e[:].rearrange("(m p) -> p m", p=P))
        bk_col = qkv.tile([P, NK], F32)
        nc.sync.dma_start(bk_col[:], bk_e[:].rearrange("(m p) -> p m", p=P))
        bv_row = qkv.tile([1, N], BF16)
        nc.sync.dma_start(bv_row[:1, :], bv_e[:].rearrange("(a n) -> a n", a=1))
        wk_sb = [qkv.tile([P, N], BF16, tag=f"wk{k}", name=f"wk{k}") for k in range(NK)]
        wq_sb = [qkv.tile([P, N], BF16, tag=f"wq{k}", name=f"wq{k}") for k in range(NK)]
        wv_sb = [qkv.tile([P, N], BF16, tag=f"wv{k}", name=f"wv{k}") for k in range(NK)]
        qT_sb = qkv.tile([P, NK * SL], BF16)
        v_sb = qkv.tile([P, NI * N], BF16)
        for k in range(NK):
            nc.sync.dma_start(wk_sb[k][:], wk_e[ts(k, P), :])
        for k in range(NK):
            # zero_bias: host passes Wq TRANSPOSED here (see kernel())
            nc.sync.dma_start(wq_sb[k][:], wq_e[ts(k, P), :])
        for k in range(NK):
            nc.sync.dma_start(wv_sb[k][:], wv_e[ts(k, P), :])

        with tc.tile_pool(name="qpsum", bufs=6, space="PSUM") as qpsum:
            # k (transposed layout, stays local)
            for m in range(NK):
                pk = qpsum.tile([P, SL], F32, tag="pq")
                for k in range(NK):
                    nc.tensor.matmul(
                        pk[:],
                        wk_sb[k][:, ts(m, P)],
                        xnT_sb[:, ts(k, SL)],
                        start=(k == 0),
                        stop=(k == NK - 1),
                    )
                nc.vector.tensor_scalar_add(kT_sb[:, ts(m, SL)], pk[:], bk_col[:, m : m + 1])

            if zero_bias:
                # kappa^T[m, i] = sum_n Wq.T[n, m] * kT[n, i]
                for m in range(NK):
                    pq = qpsum.tile([P, SL], F32, tag="pq")
                    for n in range(NK):
                        nc.tensor.matmul(
                            pq[:],
                            wq_sb[n][:, ts(m, P)],
                            kT_sb[:, ts(n, SL)],
                            start=(n == 0),
                            stop=(n == NK - 1),
                        )
                    nc.scalar.activation(rhs_sb[:, ts(m, SL)], pq[:], AF.Copy)
            else:
                # general path: q (transposed), then its all-gather
                for m in range(NK):
                    pq = qpsum.tile([P, SL], F32, tag="pq")
                    for k in range(NK):
                        nc.tensor.matmul(
                            pq[:],
                            wq_sb[k][:, ts(m, P)],
                            xnT_sb[:, ts(k, SL)],
                            start=(k == 0),
                            stop=(k == NK - 1),
                        )
                    nc.scalar.activation(
                        qT_sb[:, ts(m, SL)], pq[:], AF.Identity, bias=bq_col[:, m : m + 1]
                    )
                for m in range(NK):
                    nc.gpsimd.dma_start(agq_in[m], qT_sb[:, ts(m, SL)])
                nc.gpsimd.collective_compute(
                    "AllGather", mybir.AluOpType.bypass, replica_groups=rg,
                    ins=[agq_in[:]], outs=[agq_out[:]],
                )
                nc.vector.tensor_copy(rhs_sb[:], kT_sb[:])

            # v (natural layout) + its all-gather
            for i in range(NI):
                for cb in range(2):
                    pv = qpsum.tile([P, 512], F32, tag="pq")
                    if not zero_bias:
                        nc.tensor.matmul(
                            pv[:], ones_row_b[:], bv_row[:1, ts(cb, 512)],
                            start=True, stop=False,
                        )
                    for k in range(NK):
                        nc.tensor.matmul(
                            pv[:],
                            xnT_sb[:, k * SL + i * P : k * SL + (i + 1) * P],
                            wv_sb[k][:, ts(cb, 512)],
                            start=(zero_bias and k == 0),
                            stop=(k == NK - 1),
                        )
                    nc.vector.tensor_copy(v_sb[:, i * N + cb * 512 : i * N + (cb + 1) * 512], pv[:])
            for i in range(NI):
                nc.gpsimd.dma_start(agv_in[i], v_sb[:, ts(i, N)])
            nc.gpsimd.collective_compute(
                "AllGather", mybir.AluOpType.bypass, replica_groups=rg,
                ins=[agv_in[:]], outs=[agv_out[:]],
            )
        leave(qkv_cm)
        leave(xnT_cm)

        # W1 resident; emitted here so it prefetches during attention
        w1_cm, w1p = enter(tc.tile_pool(name="w1p", bufs=1, side="left"))
        w1_sb = [w1p.tile([P, FF], BF16, tag=f"w1{c}", name=f"w1{c}") for c in range(NK)]
        for c in range(NK):
            nc.sync.dma_start(w1_sb[c][:], w1_e[ts(c, P), :])
        b1_row = w1p.tile([1, FF], BF16)
        nc.sync.dma_start(b1_row[:1, :], b1_e[:].rearrange("(a n) -> a n", a=1))

        # =========== Phase 2: logits (transposed) + exp + running sum ===========
        wT_cm, wTp = enter(tc.tile_pool(name="wTp", bufs=1, side="left"))
        wT_sb = wTp.tile([P, NJ * SL], BF16)
        acc = wTp.tile([P, SL], F32)
        nc.vector.memset(acc[:], 0.0)
        with (
            tc.tile_pool(name="qf", bufs=6) as qfp,
            tc.tile_pool(name="wpsum", bufs=6, space="PSUM") as wpsum,
        ):
            for rank in range(R):
                qf = qfp.tile([P, NK * SL], BF16, tag="qf")
                for n in range(NK):
                    eng = nc.sync if (n + rank) % 2 == 0 else nc.scalar
                    eng.dma_start(qf[:, ts(n, SL)], agq_out[rank * NK + n])
                for sub in range(NI):
                    jc = rank * NI + sub
                    pw = wpsum.tile([P, SL], F32, tag="pw")
                    for n in range(NK):
                        nc.tensor.matmul(
                            pw[:],
                            qf[:, n * SL + sub * P : n * SL + (sub + 1) * P],
                            rhs_sb[:, ts(n, SL)],
                            start=(n == 0),
                            stop=(n == NK - 1),
                        )
                    nc.scalar.activation(
                        wT_sb[:, ts(jc, SL)], pw[:], AF.Exp, scale=float(SCALE), bias=zero_col[:]
                    )
                    nc.vector.tensor_add(acc[:], acc[:], wT_sb[:, ts(jc, SL)])
        leave(kT_cm)

        # =========== Phase 3: hT accumulation over all j ===========
        mid_cm, midp = enter(tc.tile_pool(name="midp", bufs=1, side="right"))
        hT_sb = midp.tile([P, NK * SL], BF16)
        ff1T_sb = midp.tile([P, NF * SL], BF16)
        with (
            tc.tile_pool(name="vstream", bufs=8) as vsp,
            tc.tile_pool(name="hpsum", bufs=1, space="PSUM") as hpsum,
        ):
            ph = [hpsum.tile([P, SL], F32, tag=f"ph{c}", name=f"ph{c}") for c in range(NK)]
            for j in range(NJ):
                vt = vsp.tile([P, N], BF16, tag="vt")
                (nc.sync if j < 8 else nc.gpsimd).dma_start(vt[:], agv_out[j])
                for c in range(NK):
                    nc.tensor.matmul(
                        ph[c][:],
                        vt[:, ts(c, P)],
                        wT_sb[:, ts(j, SL)],
                        start=(j == 0),
                        stop=(j == NJ - 1),
                    )
            for c in range(NK):
                if c % 2 == 0:
                    nc.scalar.activation(hT_sb[:, ts(c, SL)], ph[c][:], AF.Copy)
                else:
                    nc.vector.tensor_copy(hT_sb[:, ts(c, SL)], ph[c][:])
        # sumexp finalize: PE cost is tiny and overlaps the hT evacuations
        with tc.tile_pool(name="spsum", bufs=2, space="PSUM") as spsum:
            ps = spsum.tile([1, SL], F32, tag="ps")
            nc.tensor.matmul(ps[:], ones_col_f[:], acc[:])
            nc.vector.tensor_copy(sum_row_f[:1, :], ps[:1, :])
            if not zero_bias:
                nc.scalar.activation(sum_row_b[:1, :], ps[:1, :], AF.Copy)
            for ic in range(NI):
                pr = spsum.tile([P, 1], F32, tag="pr")
                nc.tensor.matmul(pr[:], sum_row_f[:1, ts(ic, P)], one_f[:1, :])
                nc.vector.reciprocal(recip_col[:, ic : ic + 1], pr[:])
        # deferred residual affine: xn_sb = z*g + b, done during idle DVE time
        with (
            tc.tile_pool(name="bc", bufs=1, side="left") as bc,
            tc.tile_pool(name="bpsum", bufs=2, space="PSUM") as bpsum,
        ):
            ones_row_f = bc.tile([1, P], F32)
            nc.gpsimd.memset(ones_row_f[:], 1.0)
            g_row = bc.tile([1, N], F32)
            nc.gpsimd.dma_start(g_row[:1, :], g_e[:].rearrange("(a n) -> a n", a=1))
            b_row = bc.tile([1, N], F32)
            nc.gpsimd.dma_start(b_row[:1, :], bn_e[:].rearrange("(a n) -> a n", a=1))
            g_bcast = bc.tile([P, N], F32)
            b_bcast = bc.tile([P, N], F32)
            for vec_row, bcast in ((g_row, g_bcast), (b_row, b_bcast)):
                for blk in range(2):
                    pb = bpsum.tile([P, 512], F32, tag="pb")
                    nc.tensor.matmul(pb[:], ones_row_f[:], vec_row[:1, ts(blk, 512)])
                    nc.vector.tensor_copy(bcast[:, ts(blk, 512)], pb[:])
            for i in range(NI):
                xn_i = xn_sb[:, ts(i, N)]
                nc.vector.tensor_mul(xn_i, xn_i, g_bcast[:])
                nc.vector.tensor_add(xn_i, xn_i, b_bcast[:])

        leave(wT_cm)

        # =========== Phase 4: FFN1 (transposed out, leaky via homogeneity) ===========
        with tc.tile_pool(name="fpsum", bufs=6, space="PSUM") as fpsum:
            for f in range(NF):
                pf = fpsum.tile([P, SL], F32, tag="pf")
                if not zero_bias:
                    nc.tensor.matmul(
                        pf[:], b1_row[:1, ts(f, P)], sum_row_b[:1, :],
                        start=True, stop=False,
                    )
                for c in range(NK):
                    nc.tensor.matmul(
                        pf[:],
                        w1_sb[c][:, ts(f, P)],
                        hT_sb[:, ts(c, SL)],
                        start=(zero_bias and c == 0),
                        stop=(c == NK - 1),
                    )
                nc.scalar.activation(ff1T_sb[:, ts(f, SL)], pf[:], AF.Lrelu, alpha=0.1, bias=zero_col[:])
        leave(w1_cm)

        # =========== Phase 5: FFN2 + epilogue (scale, bias, residual) ===========
        with (
            tc.tile_pool(name="ph5", bufs=1) as ph5,
            tc.tile_pool(name="w2s", bufs=8) as w2s,
            tc.tile_pool(name="outp", bufs=8) as outp,
            tc.tile_pool(name="opsum", bufs=1, space="PSUM") as opsum,
        ):
            b2_row = ph5.tile([1, N], BF16)
            nc.sync.dma_start(b2_row[:1, :], b2_e[:].rearrange("(a n) -> a n", a=1))
            po = [
                opsum.tile([P, 512], F32, tag=f"po{i}", name=f"po{i}")
                for i in range(NI * 2)
            ]
            if not zero_bias:
                for ic in range(NI):
                    for mb in range(2):
                        nc.tensor.matmul(
                            po[ic * 2 + mb][:],
                            sum_row_b[:1, ts(ic, P)],
                            b2_row[:1, ts(mb, 512)],
                            start=True, stop=False,
                        )
            # each po skips one late f-column in the main sweep; the skipped
            # column is appended per-po at the end (stop staggering) so the
            # epilogues overlap the final matmuls instead of all waiting for
            # the last one
            w2_last = [None] * NF
            for f in range(NF):
                w2t = w2s.tile([P, N], BF16, tag="w2t", name=f"w2t{f}")
                nc.scalar.dma_start(w2t[:], w2_e[ts(f, P), :])
                if f >= NF - 8:
                    w2_last[f] = w2t
                prev_loaded = None
                for g in range(NI * 2):
                    if f == NF - 8 + g:
                        continue
                    mmi = nc.tensor.matmul(
                        po[g][:],
                        ff1T_sb[:, f * SL + (g // 2) * P : f * SL + (g // 2 + 1) * P],
                        w2t[:, ts(g % 2, 512)],
                        start=(zero_bias and f == 0),
                        stop=False,
                    )
                    # consecutive mb pair shares lhsT: skip the redundant weight load
                    if prev_loaded == g // 2:
                        mmi.ins.ldweights = False
                    prev_loaded = g // 2
            for g in range(NI * 2):
                ic, mb = g // 2, g % 2
                f = NF - 8 + g
                nc.tensor.matmul(
                    po[g][:],
                    ff1T_sb[:, f * SL + ic * P : f * SL + (ic + 1) * P],
                    w2_last[f][:, ts(mb, 512)],
                    start=False,
                    stop=True,
                )
                ot = outp.tile([P, 512], F32, tag="ot")
                if g % 2 == 0:
                    nc.vector.scalar_tensor_tensor(
                        ot[:],
                        po[g][:],
                        recip_col[:, ic : ic + 1],
                        xn_sb[:, ic * N + mb * 512 : ic * N + (mb + 1) * 512],
                        op0=mybir.AluOpType.mult,
                        op1=mybir.AluOpType.add,
                    )
                else:
                    nc.scalar.activation(
                        ot[:], po[g][:], AF.Identity, scale=recip_col[:, ic : ic + 1]
                    )
                    nc.vector.tensor_add(
                        ot[:], ot[:], xn_sb[:, ic * N + mb * 512 : ic * N + (mb + 1) * 512]
                    )
                oeng = (nc.sync, nc.scalar, nc.gpsimd)[g % 3]
                oeng.dma_start(out_e[ts(ic, P), ts(mb, 512)], ot[:])
        leave(mid_cm)
        leave(base_cm)

    nc.compile()
    return nc


def _get_nc(key):
    global _cached
    if _cached is None:
        _cached = {}
    if key not in _cached:
        if key == "fp8":
            _cached[key] = _build_fp8()
        else:
            _cached[key] = _build(key)
    return _cached[key]


def kernel(**inputs):
    zero_bias = all(
        not np.any(np.asarray(inputs[k], dtype=np.float32))
        for k in ("bq", "bk", "bv", "b1", "b2")
    )
    unit_affine = (
        np.all(np.asarray(inputs["norm_g"], dtype=np.float32) == 1.0)
        and not np.any(np.asarray(inputs["norm_b"], dtype=np.float32))
    )
    if zero_bias and unit_affine:
        nc = _get_nc("fp8")
        in_maps = _fp8_in_maps(inputs)
        res = run_bass_kernel_spmd(nc, in_maps, list(range(R)))
        return np.concatenate([res.results[r]["out"] for r in range(R)], axis=0)

    nc = _get_nc(zero_bias)
    bf = lambda a: np.asarray(a, dtype=np.float32).astype(ml_dtypes.bfloat16)
    f = lambda a: np.ascontiguousarray(np.asarray(a, dtype=np.float32))
    x = f(inputs["x"])
    common = {
        "norm_g": f(inputs["norm_g"]),
        "norm_b": f(inputs["norm_b"]),
        "wq": bf(np.ascontiguousarray(np.asarray(inputs["Wq"]).T)) if zero_bias else bf(inputs["Wq"]),
        "bq": f(inputs["bq"]),
        "wk": bf(inputs["Wk"]),
        "bk": f(inputs["bk"]),
        "wv": bf(inputs["Wv"]),
        "bv": bf(inputs["bv"]),
        "w1": bf(inputs["W1"]),
        "b1": bf(inputs["b1"]),
        "w2": bf(inputs["W2"]),
        "b2": bf(inputs["b2"]),
    }
    in_maps = [dict(common, x=np.ascontiguousarray(x[r * SL : (r + 1) * SL])) for r in range(R)]
    res = run_bass_kernel_spmd(nc, in_maps, list(range(R)))
    return np.concatenate([res.results[r]["out"] for r in range(R)], axis=0)


if __name__ == "__main__":
    rng = np.random.default_rng(0)
    demo = {
        "x": rng.standard_normal((S, N), dtype=np.float32),
        "norm_g": np.ones(N, np.float32),
        "norm_b": np.zeros(N, np.float32),
        "Wq": rng.standard_normal((N, N), dtype=np.float32) * SCALE,
        "bq": np.zeros(N, np.float32),
        "Wk": rng.standard_normal((N, N), dtype=np.float32) * SCALE,
        "bk": np.zeros(N, np.float32),
        "Wv": rng.standard_normal((N, N), dtype=np.float32) * SCALE,
        "bv": np.zeros(N, np.float32),
        "W1": rng.standard_normal((N, FF), dtype=np.float32) * SCALE,
        "b1": np.zeros(FF, np.float32),
        "W2": rng.standard_normal((FF, N), dtype=np.float32) * (1.0 / np.sqrt(FF)),
        "b2": np.zeros(N, np.float32),
    }
    out = kernel(**demo)
    print("out", out.shape, out.dtype, np.abs(out).mean())


# revision 23
# speedup vs baseline: 1.1106x; 1.1106x over previous
"""Distributed transformer-block kernel for one TRN2 chip (8 NeuronCores).

Reference computation (S=4096, N=1024):
    xn = LayerNorm(x) * g + b
    q,k,v = xn@Wq+bq, xn@Wk+bk, xn@Wv+bv
    w = softmax((k @ q.T) / sqrt(N), axis=-1)
    h = w @ v
    out = leaky_relu(h@W1+b1, 0.1) @ W2 + b2 + xn

Sharding: sequence-parallel. Each core owns 512 rows of x, computes its
q/k/v shard, all-gathers xn and v, then computes its 512-row slice of
attention + FFN fully locally.

Fast path (zero biases, unit LN affine — the shipped problem): all five
matmul groups run in fp8(e4m3) DoubleRow mode (K=256 per instruction,
2x PE throughput). Weights are host-packed into row-pair layout and
pre-scaled by 32/64 so fp8 sees ~unit-sigma values; every rescale is an
exact power of 2, compensated in the exp scale/bias and the final
1/sumexp epilogue (leaky_relu is positively homogeneous):
    host: wk,wq,wv,w1 x32; w2 x64
    kT/kappa/v evac x2^-5  -> true-scale fp8
    exp bias -ln4          -> wT' = exp(logit)/4 (max ~67 < 240 e4m3 max)
    hT evac x2^-6          -> hT' = hT_un * 2^-8
    ff1 evac x1/4 -> 2^-5 ff1_un ; ff2 psum = 2 ff2_un
    sum_row = 8*sume' = 2*sume ;  out = po/(8*sume') + xn

Collectives are fp8 and split in halves (xnT by column-half, v by
row-pair) so transfers pipeline with the projections and are consumed
in arrival order (logits sub 0/1 then 2/3; h even pairs then odd).

Logits computed TRANSPOSED with Wq folded into the k side:
wT[j,i] = xn_full[j] . (Wq @ k_loc[i]), so the gather operand (xn) is
ready right after LayerNorm and feeds the collective early.
"""

import sys

sys.path.insert(0, "/opt/trn_rl_repo")

import numpy as np
import ml_dtypes

import concourse.bass as bass
from concourse import bacc, tile, mybir
from concourse.bass import ts
from concourse.bass_utils import run_bass_kernel_spmd
from concourse.masks import make_identity

F32 = mybir.dt.float32
BF16 = mybir.dt.bfloat16
F8 = mybir.dt.float8e4
AF = mybir.ActivationFunctionType
DR = mybir.MatmulPerfMode.DoubleRow

P = 128
R = 8            # cores
S = 4096         # sequence
N = 1024         # hidden
FF = 4096        # ffn hidden
SL = S // R      # local rows (512)
HSL = SL // 2    # 256
NK = N // P      # 8 hidden chunks
NKH = NK // 2    # 4 hidden pair-chunks
NI = SL // P     # 4 local row chunks
NJ = S // P      # 32 global row chunks
NF = FF // P     # 32 ffn chunks
NFH = NF // 2    # 16 ffn pair-chunks
SCALE = 1.0 / np.sqrt(N).astype(np.float32)  # 0.03125
EPS = 1e-5
NEG_LN4 = float(-np.log(4.0))

_cached = None
_DBG = False


def _build_fp8():
    nc = bacc.Bacc("TRN2", target_bir_lowering=False, debug=False, num_devices=R)

    x_e = nc.declare_dram_parameter("x", [SL, N], F32, isOutput=False)
    wkp_e = nc.declare_dram_parameter("wkp", [N // 2, 2 * N], F8, isOutput=False)
    wqp_e = nc.declare_dram_parameter("wqp", [N // 2, 2 * N], F8, isOutput=False)
    wvp_e = nc.declare_dram_parameter("wvp", [N // 2, 2 * N], F8, isOutput=False)
    w1p_e = nc.declare_dram_parameter("w1p", [N // 2, 2 * FF], F8, isOutput=False)
    w2p_e = nc.declare_dram_parameter("w2p", [FF // 2, 2 * N], F8, isOutput=False)
    out_e = nc.declare_dram_parameter("out", [SL, N], F32, isOutput=True)
    if _DBG:
        d_vt = nc.declare_dram_parameter("d_vt", [2 * R, P, 2 * N], F32, isOutput=True)
        d_acc = nc.declare_dram_parameter("d_acc", [P, SL], F32, isOutput=True)
        d_hT = nc.declare_dram_parameter("d_hT", [NK, P, SL], F32, isOutput=True)
        d_ff1 = nc.declare_dram_parameter("d_ff1", [4, P, SL], F32, isOutput=True)
        d_rec = nc.declare_dram_parameter("d_rec", [P, NI], F32, isOutput=True)
        d_wTs = nc.declare_dram_parameter("d_wTs", [4, P, SL], F32, isOutput=True)

    # collective bounce buffers (fp8), split in halves for pipelining
    agq_in = nc.dram_tensor("agq_in", [NK, P, SL], F8)
    agq_out = nc.dram_tensor("agq_out", [R * NK, P, SL], F8, addr_space="Shared")
    agv_in_a = nc.dram_tensor("agv_in_a", [2, P, N], F8)
    agv_in_b = nc.dram_tensor("agv_in_b", [2, P, N], F8)
    agv_out_a = nc.dram_tensor("agv_out_a", [R * 2, P, N], F8, addr_space="Shared")
    agv_out_b = nc.dram_tensor("agv_out_b", [R * 2, P, N], F8, addr_space="Shared")
    dum_in = nc.dram_tensor("dum_in", [1, P, 1], F32)
    dum_out = nc.dram_tensor("dum_out", [R, P, 1], F32, addr_space="Shared")

    rg = [list(range(R))]

    def enter(cm):
        return cm, cm.__enter__()

    def leave(cm):
        cm.__exit__(None, None, None)

    with tile.TileContext(nc) as tc:
        base_cm, basep = enter(tc.tile_pool(name="base", bufs=1))

        ident = basep.tile([P, P], BF16)
        make_identity(nc, ident)
        ones_col_f = basep.tile([P, 1], F32)
        nc.gpsimd.memset(ones_col_f[:], 1.0)
        one_f = basep.tile([1, 1], F32)
        nc.gpsimd.memset(one_f[:], 1.0)
        zero_col = basep.tile([P, 1], F32)
        nc.gpsimd.memset(zero_col[:], 0.0)
        eps_col = basep.tile([P, 1], F32)
        nc.gpsimd.memset(eps_col[:], EPS)
        nln4_col = basep.tile([P, 1], F32)
        nc.gpsimd.memset(nln4_col[:], NEG_LN4)

        xn_sb = basep.tile([P, NI * N], BF16)    # normed x, natural (residual)
        sum_row_f = basep.tile([1, SL], F32)
        recip_col = basep.tile([P, NI], F32)

        # x loads first (LayerNorm is the critical path at startup), then
        # the resident fp8 weights behind them on the same sync queue
        wts_cm, wts = enter(tc.tile_pool(name="wts", bufs=1, side="left"))
        wkp = [wts.tile([P, 2, N], F8, tag=f"wkp{i}", name=f"wkp{i}") for i in range(NKH)]
        wqp = [wts.tile([P, 2, N], F8, tag=f"wqp{i}", name=f"wqp{i}") for i in range(NKH)]
        wvp = [wts.tile([P, 2, N], F8, tag=f"wvp{i}", name=f"wvp{i}") for i in range(NKH)]
        w1_cm, w1pool = enter(tc.tile_pool(name="w1pool", bufs=1, side="left"))
        w1p = [w1pool.tile([P, 2, FF], F8, tag=f"w1p{i}", name=f"w1p{i}") for i in range(NKH)]
        xs_cm, xs = enter(tc.tile_pool(name="xs", bufs=1))
        xts = [xs.tile([P, N], F32, tag=f"xt{i}", name=f"xt{i}") for i in range(NI)]
        for i in range(NI):
            (nc.sync if i % 2 == 0 else nc.scalar).dma_start(xts[i][:], x_e[ts(i, P), :])
        for i in range(NKH):
            nc.sync.dma_start(wkp[i][:], wkp_e[ts(i, P), :].rearrange("p (two n) -> p two n", two=2))
        for i in range(NKH):
            nc.sync.dma_start(wqp[i][:], wqp_e[ts(i, P), :].rearrange("p (two n) -> p two n", two=2))
        for i in range(NKH):
            nc.sync.dma_start(wvp[i][:], wvp_e[ts(i, P), :].rearrange("p (two n) -> p two n", two=2))
        for i in range(NKH):
            nc.sync.dma_start(w1p[i][:], w1p_e[ts(i, P), :].rearrange("p (two n) -> p two n", two=2))

        # =========== Phase 0: layernorm + transpose ===========
        xnT_cm, xnTp = enter(tc.tile_pool(name="xnTp", bufs=1, side="left"))
        xnT_sb = xnTp.tile([P, NK, SL], F8)

        with (
            tc.tile_pool(name="ln", bufs=4) as ln,
            tc.tile_pool(name="lnsq", bufs=2) as lnsq,
            tc.tile_pool(name="tpsum", bufs=8, space="PSUM") as tpsum,
        ):
            for i in range(NI):
                xt = xts[i]
                sum_t = ln.tile([P, 1], F32, tag="sum")
                nc.vector.reduce_sum(sum_t[:], xt[:], axis=mybir.AxisListType.X)
                sq_scr = lnsq.tile([P, N], BF16, tag="sq")
                sumsq_t = ln.tile([P, 1], F32, tag="sumsq")
                nc.scalar.activation(sq_scr[:], xt[:], AF.Square, bias=zero_col[:], accum_out=sumsq_t[:])
                mu_t = ln.tile([P, 1], F32, tag="mu")
                nc.gpsimd.tensor_scalar_mul(mu_t[:], sum_t[:], 1.0 / N)
                var_t = ln.tile([P, 1], F32, tag="var")
                nc.gpsimd.tensor_scalar_mul(var_t[:], sumsq_t[:], 1.0 / N)
                musq_t = ln.tile([P, 1], F32, tag="musq")
                nc.gpsimd.tensor_mul(musq_t[:], mu_t[:], mu_t[:])
                nc.gpsimd.tensor_sub(var_t[:], var_t[:], musq_t[:])
                std_t = ln.tile([P, 1], F32, tag="std")
                nc.scalar.activation(std_t[:], var_t[:], AF.Sqrt, bias=eps_col[:])
                rstd_t = ln.tile([P, 1], F32, tag="rstd")
                nc.vector.reciprocal(rstd_t[:], std_t[:])
                nmr_t = ln.tile([P, 1], F32, tag="nmr")
                nc.gpsimd.tensor_mul(nmr_t[:], mu_t[:], rstd_t[:])
                nc.gpsimd.tensor_scalar_mul(nmr_t[:], nmr_t[:], -1.0)
                # xn_sb = (x-mu)*rstd : with g=1,b=0 this is final
                xn_i = xn_sb[:, ts(i, N)]
                nc.scalar.activation(xn_i, xt[:], AF.Identity, scale=rstd_t[:], bias=nmr_t[:])
                for k in range(NK):
                    pt = tpsum.tile([P, P], BF16, tag="pt")
                    nc.tensor.transpose(pt[:], xn_sb[:, i * N + k * P : i * N + (k + 1) * P], ident[:])
                    nc.vector.tensor_copy(xnT_sb[:, k, ts(i, P)], pt[:])

        # =========== Phase 1: gathers + projections ===========
        # one xnT gather: logits consume it only after the local projections,
        # so splitting it buys nothing but extra mesh overhead
        nc.gpsimd.dma_start(agq_in[:].rearrange("k p s -> p k s"), xnT_sb[:, :, :])
        nc.gpsimd.collective_compute(
            "AllGather", mybir.AluOpType.bypass, replica_groups=rg,
            ins=[agq_in[:]], outs=[agq_out[:]],
        )

        kv_cm, kvp = enter(tc.tile_pool(name="kvp", bufs=1, side="right"))
        kT_sb = kvp.tile([P, NK, SL], F8)
        rhs_sb = kvp.tile([P, NK, SL], F8)   # kappa = Wq @ k, fp8
        v_sb = kvp.tile([P, NI, N], F8)

        with tc.tile_pool(name="qpsum", bufs=6, space="PSUM") as qpsum:
            # kT[m, i] = sum_n Wk[n, m] xnT[n, i]
            for m in range(NK):
                pk = qpsum.tile([P, SL], F32, tag="pq")
                for pi in range(NKH):
                    nc.tensor.matmul(
                        pk[:],
                        wkp[pi][:, :, ts(m, P)],
                        xnT_sb[:, 2 * pi : 2 * pi + 2, :],
                        start=(pi == 0), stop=(pi == NKH - 1), perf_mode=DR,
                    )
                nc.vector.tensor_scalar_mul(kT_sb[:, m, :], pk[:], 2.0 ** -5)
            # kappa[m, i] = sum_n Wq[m, n] kT[n, i]
            for m in range(NK):
                pq = qpsum.tile([P, SL], F32, tag="pq")
                for pi in range(NKH):
                    nc.tensor.matmul(
                        pq[:],
                        wqp[pi][:, :, ts(m, P)],
                        kT_sb[:, 2 * pi : 2 * pi + 2, :],
                        start=(pi == 0), stop=(pi == NKH - 1), perf_mode=DR,
                    )
                nc.vector.tensor_scalar_mul(rhs_sb[:, m, :], pq[:], 2.0 ** -5)
            # v[i, c] = sum_n xnT[n, i] Wv[n, c]; cb-inner shares the
            # stationary xnT block so the second matmul skips ldweights
            for i in range(NI):
                pv = [qpsum.tile([P, 512], F32, tag="pq", name=f"pv{i}_{cb}") for cb in range(2)]
                for pi in range(NKH):
                    for cb in range(2):
                        mmi = nc.tensor.matmul(
                            pv[cb][:],
                            xnT_sb[:, 2 * pi : 2 * pi + 2, ts(i, P)],
                            wvp[pi][:, :, ts(cb, 512)],
                            start=(pi == 0), stop=(pi == NKH - 1), perf_mode=DR,
                        )
                        if cb == 1:
                            mmi.ins.ldweights = False
                for cb in range(2):
                    nc.vector.tensor_scalar_mul(v_sb[:, i, ts(cb, 512)], pv[cb][:], 2.0 ** -5)
                # launch v-gather halves as soon as their chunks are done
                if i == 1:
                    nc.gpsimd.dma_start(agv_in_a[:].rearrange("k p n -> p k n"), v_sb[:, 0:2, :])
                    nc.gpsimd.collective_compute(
                        "AllGather", mybir.AluOpType.bypass, replica_groups=rg,
                        ins=[agv_in_a[:]], outs=[agv_out_a[:]],
                    )
                if i == 3:
                    nc.gpsimd.dma_start(agv_in_b[:].rearrange("k p n -> p k n"), v_sb[:, 2:4, :])
                    nc.gpsimd.collective_compute(
                        "AllGather", mybir.AluOpType.bypass, replica_groups=rg,
                        ins=[agv_in_b[:]], outs=[agv_out_b[:]],
                    )
        leave(xnT_cm)
        leave(xs_cm)

        # =========== Phase 2: logits (transposed) + exp + running sum ===========
        wT_cm, wTp = enter(tc.tile_pool(name="wTp", bufs=1, side="left"))
        wT_sb = wTp.tile([P, NJ, SL], F8)
        acc = wTp.tile([P, SL], F32)
        nc.vector.memset(acc[:], 0.0)
        # whole gathered v kept resident (4MB fp8); loads overlap phase 2
        vt_cm, vtp = enter(tc.tile_pool(name="vtp", bufs=1, side="left"))
        vt = [vtp.tile([P, 2, N], F8, tag=f"vt{b}", name=f"vt{b}") for b in range(2 * R)]
        with (
            tc.tile_pool(name="qf", bufs=3) as qfp,
            tc.tile_pool(name="wpsum", bufs=3, space="PSUM") as wpsum,
        ):
            def logits_chunk(qf, jc, sub_in_half):
                pw = wpsum.tile([P, SL], F32, tag="pw")
                for pi in range(NKH):
                    nc.tensor.matmul(
                        pw[:],
                        qf[:, 2 * pi : 2 * pi + 2, ts(sub_in_half, P)],
                        rhs_sb[:, 2 * pi : 2 * pi + 2, :],
                        start=(pi == 0), stop=(pi == NKH - 1), perf_mode=DR,
                    )
                nc.scalar.activation(
                    wT_sb[:, jc, :], pw[:], AF.Exp, scale=float(SCALE), bias=nln4_col[:]
                )
                nc.vector.tensor_add(acc[:], acc[:], wT_sb[:, jc, :])

            for rank in range(R):
                qf = qfp.tile([P, NK, SL], F8, tag="qf")
                nc.sync.dma_start(qf[:], agq_out[rank * NK : (rank + 1) * NK].rearrange("k p s -> p k s"))
                for sub in range(NI):
                    logits_chunk(qf, rank * NI + sub, sub)
                nc.gpsimd.dma_start(vt[2 * rank][:], agv_out_a[2 * rank : 2 * rank + 2].rearrange("k p n -> p k n"))
            # odd vt tiles gate on the second v-gather; keep them BEHIND all
            # even loads so they don't head-of-line block the h phase
            for rank in range(R):
                nc.gpsimd.dma_start(vt[2 * rank + 1][:], agv_out_b[2 * rank : 2 * rank + 2].rearrange("k p n -> p k n"))
        leave(kv_cm)

        # =========== Phase 3: hT accumulation ===========
        mid_cm, midp = enter(tc.tile_pool(name="midp", bufs=1, side="right"))
        hT_sb = midp.tile([P, NK, SL], F8)
        ff1T_sb = midp.tile([P, NF, SL], F8)
        # w2 resident; loads overlap attention
        w2_cm, w2pool = enter(tc.tile_pool(name="w2pool", bufs=1, side="right"))
        w2p = [w2pool.tile([P, 2, N], F8, tag=f"w2p{t}", name=f"w2p{t}") for t in range(NFH)]
        for t in range(NFH):
            nc.sync.dma_start(
                w2p[t][:], w2p_e[ts(t, P), :].rearrange("p (two n) -> p two n", two=2)
            )
        with tc.tile_pool(name="hpsum", bufs=1, space="PSUM") as hpsum:
            ph = [hpsum.tile([P, SL], F32, tag=f"ph{c}", name=f"ph{c}") for c in range(NK)]
            border = [2 * r for r in range(R)] + [2 * r + 1 for r in range(R)]
            for step, b in enumerate(border):
                for c in range(NK):
                    nc.tensor.matmul(
                        ph[c][:],
                        vt[b][:, :, ts(c, P)],
                        wT_sb[:, 2 * b : 2 * b + 2, :],
                        start=(step == 0), stop=(step == 2 * R - 1), perf_mode=DR,
                    )
            for c in range(NK):
                if c % 2 == 0:
                    nc.scalar.activation(hT_sb[:, c, :], ph[c][:], AF.Identity, scale=2.0 ** -6, bias=zero_col[:])
                else:
                    nc.vector.tensor_scalar_mul(hT_sb[:, c, :], ph[c][:], 2.0 ** -6)
        if _DBG:
            with tc.tile_pool(name="dbgp", bufs=2) as dbgp:
                for b in range(2 * R):
                    vtf = dbgp.tile([P, 2 * N], F32, tag="dbgv")
                    nc.vector.tensor_copy(vtf[:], vt[b][:].rearrange("p two n -> p (two n)"))
                    nc.sync.dma_start(d_vt[b], vtf[:])
                nc.sync.dma_start(d_acc[:], acc[:])
                for c in range(NK):
                    htf = dbgp.tile([P, SL], F32, tag="dbgs")
                    nc.vector.tensor_copy(htf[:], hT_sb[:, c, :])
                    nc.sync.dma_start(d_hT[c], htf[:])
                for jc in range(4):
                    wtf = dbgp.tile([P, SL], F32, tag="dbgs")
                    nc.vector.tensor_copy(wtf[:], wT_sb[:, 16 + jc, :])
                    nc.sync.dma_start(d_wTs[jc], wtf[:])
        # sumexp finalize: sum over partitions, then 1/(32*sume')
        with tc.tile_pool(name="spsum", bufs=2, space="PSUM") as spsum:
            ps = spsum.tile([1, SL], F32, tag="ps")
            nc.tensor.matmul(ps[:], ones_col_f[:], acc[:])
            nc.vector.tensor_scalar_mul(sum_row_f[:1, :], ps[:1, :], 8.0)
            for ic in range(NI):
                pr = spsum.tile([P, 1], F32, tag="pr")
                nc.tensor.matmul(pr[:], sum_row_f[:1, ts(ic, P)], one_f[:1, :])
                nc.vector.reciprocal(recip_col[:, ic : ic + 1], pr[:])
        leave(vt_cm)
        leave(wT_cm)

        # =========== Phase 4: FFN1 ===========
        with tc.tile_pool(name="fpsum", bufs=3, space="PSUM") as fpsum:
            for f in range(NF):
                pf = fpsum.tile([P, SL], F32, tag="pf")
                for pi in range(NKH):
                    nc.tensor.matmul(
                        pf[:],
                        w1p[pi][:, :, ts(f, P)],
                        hT_sb[:, 2 * pi : 2 * pi + 2, :],
                        start=(pi == 0), stop=(pi == NKH - 1), perf_mode=DR,
                    )
                nc.scalar.activation(ff1T_sb[:, f, :], pf[:], AF.Lrelu, alpha=0.1, scale=0.25, bias=zero_col[:])
        leave(w1_cm)
        leave(wts_cm)
        if _DBG:
            with tc.tile_pool(name="dbgp2", bufs=2) as dbgp2:
                for f in range(4):
                    f1f = dbgp2.tile([P, SL], F32, tag="dbgs2")
                    nc.vector.tensor_copy(f1f[:], ff1T_sb[:, f, :])
                    nc.sync.dma_start(d_ff1[f], f1f[:])
                nc.sync.dma_start(d_rec[:], recip_col[:])

        # =========== Phase 5: FFN2 + epilogue ===========
        with (
            tc.tile_pool(name="outp", bufs=8) as outp,
            tc.tile_pool(name="opsum", bufs=1, space="PSUM") as opsum,
        ):
            po = [opsum.tile([P, 512], F32, tag=f"po{g}", name=f"po{g}") for g in range(NI * 2)]
            # bank g skips step 8+g in the main sweep; appended at the end
            # (stop staggering) so epilogues overlap the final matmuls
            for t in range(NFH):
                prev_loaded = None
                for g in range(NI * 2):
                    if t == NFH - 8 + g:
                        continue
                    ic, mb = g // 2, g % 2
                    mmi = nc.tensor.matmul(
                        po[g][:],
                        ff1T_sb[:, 2 * t : 2 * t + 2, ts(ic, P)],
                        w2p[t][:, :, ts(mb, 512)],
                        start=(t == 0), stop=False, perf_mode=DR,
                    )
                    if prev_loaded == (t, ic):
                        mmi.ins.ldweights = False
                    prev_loaded = (t, ic)
            for g in range(NI * 2):
                ic, mb = g // 2, g % 2
                t = NFH - 8 + g
                nc.tensor.matmul(
                    po[g][:],
                    ff1T_sb[:, 2 * t : 2 * t + 2, ts(ic, P)],
                    w2p[t][:, :, ts(mb, 512)],
                    start=False, stop=True, perf_mode=DR,
                )
                ot = outp.tile([P, 512], F32, tag="ot")
                if g % 2 == 0:
                    nc.vector.scalar_tensor_tensor(
                        ot[:], po[g][:], recip_col[:, ic : ic + 1],
                        xn_sb[:, ic * N + mb * 512 : ic * N + (mb + 1) * 512],
                        op0=mybir.AluOpType.mult, op1=mybir.AluOpType.add,
                    )
                else:
                    nc.scalar.activation(ot[:], po[g][:], AF.Identity, scale=recip_col[:, ic : ic + 1])
                    nc.vector.tensor_add(
                        ot[:], ot[:], xn_sb[:, ic * N + mb * 512 : ic * N + (mb + 1) * 512]
                    )
                oeng = (nc.sync, nc.scalar, nc.gpsimd)[g % 3]
                oeng.dma_start(out_e[ts(ic, P), ts(mb, 512)], ot[:])
        leave(w2_cm)
        leave(mid_cm)
        leave(base_cm)

    nc.compile()
    return nc


def _packpair(W):
    """[K, M] -> [K//2, 2M]: row-pair layout for DoubleRow lhsT/rhs tiles."""
    K, M = W.shape
    return np.ascontiguousarray(
        W.reshape(K // 256, 2, 128, M).transpose(0, 2, 1, 3).reshape(K // 2, 2 * M)
    )


def _fp8_in_maps(inputs):
    p8 = lambda a: np.asarray(a, dtype=np.float32).astype(ml_dtypes.float8_e4m3)
    f = lambda a: np.ascontiguousarray(np.asarray(a, dtype=np.float32))
    Wq = np.asarray(inputs["Wq"], dtype=np.float32)
    Wk = np.asarray(inputs["Wk"], dtype=np.float32)
    Wv = np.asarray(inputs["Wv"], dtype=np.float32)
    W1 = np.asarray(inputs["W1"], dtype=np.float32)
    W2 = np.asarray(inputs["W2"], dtype=np.float32)
    common = {
        "wkp": p8(_packpair(Wk * 32.0)),
        "wqp": p8(_packpair(np.ascontiguousarray(Wq.T) * 32.0)),
        "wvp": p8(_packpair(Wv * 32.0)),
        "w1p": p8(_packpair(W1 * 32.0)),
        "w2p": p8(_packpair(W2 * 64.0)),
    }
    x = f(inputs["x"])
    return [dict(common, x=np.ascontiguousarray(x[r * SL : (r + 1) * SL])) for r in range(R)]


# ---------------------------------------------------------------------------
# General (bf16) fallback for nonzero biases / non-unit LN affine.
# ---------------------------------------------------------------------------

def _build(zero_bias):
    nc = bacc.Bacc("TRN2", target_bir_lowering=False, debug=False, num_devices=R)

    x_e = nc.declare_dram_parameter("x", [SL, N], F32, isOutput=False)
    g_e = nc.declare_dram_parameter("norm_g", [N], F32, isOutput=False)
    bn_e = nc.declare_dram_parameter("norm_b", [N], F32, isOutput=False)
    wq_e = nc.declare_dram_parameter("wq", [N, N], BF16, isOutput=False)
    bq_e = nc.declare_dram_parameter("bq", [N], F32, isOutput=False)
    wk_e = nc.declare_dram_parameter("wk", [N, N], BF16, isOutput=False)
    bk_e = nc.declare_dram_parameter("bk", [N], F32, isOutput=False)
    wv_e = nc.declare_dram_parameter("wv", [N, N], BF16, isOutput=False)
    bv_e = nc.declare_dram_parameter("bv", [N], BF16, isOutput=False)
    w1_e = nc.declare_dram_parameter("w1", [N, FF], BF16, isOutput=False)
    b1_e = nc.declare_dram_parameter("b1", [FF], BF16, isOutput=False)
    w2_e = nc.declare_dram_parameter("w2", [FF, N], BF16, isOutput=False)
    b2_e = nc.declare_dram_parameter("b2", [N], BF16, isOutput=False)
    out_e = nc.declare_dram_parameter("out", [SL, N], F32, isOutput=True)

    # collective bounce buffers
    agq_in = nc.dram_tensor("agq_in", [NK, P, SL], BF16)
    agq_out = nc.dram_tensor("agq_out", [R * NK, P, SL], BF16, addr_space="Shared")
    agv_in = nc.dram_tensor("agv_in", [NI, P, N], BF16)
    agv_out = nc.dram_tensor("agv_out", [R * NI, P, N], BF16, addr_space="Shared")

    rg = [list(range(R))]

    def enter(cm):
        return cm, cm.__enter__()

    def leave(cm):
        cm.__exit__(None, None, None)

    with tile.TileContext(nc) as tc:
        base_cm, base = enter(tc.tile_pool(name="base", bufs=1))

        # ---- whole-kernel constants / carriers ----
        ident = base.tile([P, P], BF16)
        make_identity(nc, ident)
        ones_row_b = base.tile([1, P], BF16)
        nc.gpsimd.memset(ones_row_b[:], 1.0)
        ones_col_f = base.tile([P, 1], F32)
        nc.gpsimd.memset(ones_col_f[:], 1.0)
        one_f = base.tile([1, 1], F32)
        nc.gpsimd.memset(one_f[:], 1.0)
        zero_col = base.tile([P, 1], F32)
        nc.gpsimd.memset(zero_col[:], 0.0)
        eps_col = base.tile([P, 1], F32)
        nc.gpsimd.memset(eps_col[:], EPS)

        xn_sb = base.tile([P, NI * N], BF16)    # normed x, natural layout (residual)
        sum_row_f = base.tile([1, SL], F32)
        sum_row_b = base.tile([1, SL], BF16)
        recip_col = base.tile([P, NI], F32)

        # =========== Phase 0: layernorm + transpose ===========
        xnT_cm, xnTp = enter(tc.tile_pool(name="xnTp", bufs=1, side="left"))
        xnT_sb = xnTp.tile([P, NK * SL], BF16)

        # per-partition views of the LN affine for the transposed layout
        g_col = base.tile([P, NK], F32)
        nc.sync.dma_start(g_col[:], g_e[:].rearrange("(m p) -> p m", p=P))
        b_col = base.tile([P, NK], F32)
        nc.sync.dma_start(b_col[:], bn_e[:].rearrange("(m p) -> p m", p=P))

        with (
            tc.tile_pool(name="xs", bufs=4) as xs,
            tc.tile_pool(name="ln", bufs=4) as ln,
            tc.tile_pool(name="tpsum", bufs=8, space="PSUM") as tpsum,
        ):
            for i in range(NI):
                xt = xs.tile([P, N], F32, tag="xt")
                nc.sync.dma_start(xt[:], x_e[ts(i, P), :])
                sum_t = ln.tile([P, 1], F32, tag="sum")
                nc.vector.reduce_sum(sum_t[:], xt[:], axis=mybir.AxisListType.X)
                sq_scr = lnsq.tile([P, N], BF16, tag="sq")
                sumsq_t = ln.tile([P, 1], F32, tag="sumsq")
                nc.scalar.activation(sq_scr[:], xt[:], AF.Square, bias=zero_col[:], accum_out=sumsq_t[:])
                mu_t = ln.tile([P, 1], F32, tag="mu")
                nc.gpsimd.tensor_scalar_mul(mu_t[:], sum_t[:], 1.0 / N)
                var_t = ln.tile([P, 1], F32, tag="var")
                nc.gpsimd.tensor_scalar_mul(var_t[:], sumsq_t[:], 1.0 / N)
                musq_t = ln.tile([P, 1], F32, tag="musq")
                nc.gpsimd.tensor_mul(musq_t[:], mu_t[:], mu_t[:])
                nc.gpsimd.tensor_sub(var_t[:], var_t[:], musq_t[:])
                std_t = ln.tile([P, 1], F32, tag="std")
                nc.scalar.activation(std_t[:], var_t[:], AF.Sqrt, bias=eps_col[:])
                rstd_t = ln.tile([P, 1], F32, tag="rstd")
                nc.vector.reciprocal(rstd_t[:], std_t[:])
                nmr_t = ln.tile([P, 1], F32, tag="nmr")
                nc.gpsimd.tensor_mul(nmr_t[:], mu_t[:], rstd_t[:])
                nc.gpsimd.tensor_scalar_mul(nmr_t[:], nmr_t[:], -1.0)
                # xn_sb holds z = (x-mu)*rstd (bf16); affine for the residual
                # is applied in-place later, off the critical path
                xn_i = xn_sb[:, ts(i, N)]
                nc.scalar.activation(xn_i, xt[:], AF.Identity, scale=rstd_t[:], bias=nmr_t[:])
                for k in range(NK):
                    pt = tpsum.tile([P, P], BF16, tag="pt")
                    nc.tensor.transpose(pt[:], xn_sb[:, i * N + k * P : i * N + (k + 1) * P], ident[:])
                    # affine fused here: in transposed layout g,b are per-partition
                    nc.scalar.activation(
                        xnT_sb[:, k * SL + i * P : k * SL + (i + 1) * P], pt[:], AF.Identity,
                        scale=g_col[:, k : k + 1], bias=b_col[:, k : k + 1],
                    )

        # =========== Phase 1: projections + all-gathers ===========
        # zero_bias path: gather xnT itself (ready far earlier than q), and
        # fold Wq into the k side:  logits = xnT_full . (Wq @ kT)  — same
        # matmul count, but the collective launches ~35us sooner.
        kT_cm, kTp = enter(tc.tile_pool(name="kTp", bufs=1, side="right"))
        kT_sb = kTp.tile([P, NK * SL], BF16)
        rhs_sb = kTp.tile([P, NK * SL], BF16)  # logits rhs: kappa^T (zero_bias) or kT

        if zero_bias:
            for m in range(NK):
                (nc.gpsimd if m % 2 == 0 else nc.scalar).dma_start(agq_in[m], xnT_sb[:, ts(m, SL)])
            nc.gpsimd.collective_compute(
                "AllGather", mybir.AluOpType.bypass, replica_groups=rg,
                ins=[agq_in[:]], outs=[agq_out[:]],
            )

        qkv_cm, qkv = enter(tc.tile_pool(name="qkv", bufs=1, side="right"))
        bq_col = qkv.tile([P, NK], F32)
        nc.sync.dma_start(bq_col[:], bq_e[:].rearrange("(m p) -> p m", p=P))
        bk_col = qkv.tile([P, NK], F32)
        nc.sync.dma_start(bk_col[:], bk_e[:].rearrange("(m p) -> p m", p=P))
        bv_row = qkv.tile([1, N], BF16)
        nc.sync.dma_start(bv_row[:1, :], bv_e[:].rearrange("(a n) -> a n", a=1))
        wk_sb = [qkv.tile([P, N], BF16, tag=f"wk{k}", name=f"wk{k}") for k in range(NK)]
        wq_sb = [qkv.tile([P, N], BF16, tag=f"wq{k}", name=f"wq{k}") for k in range(NK)]
        wv_sb = [qkv.tile([P, N], BF16, tag=f"wv{k}", name=f"wv{k}") for k in range(NK)]
        qT_sb = qkv.tile([P, NK * SL], BF16)
        v_sb = qkv.tile([P, NI * N], BF16)
        for k in range(NK):
            nc.sync.dma_start(wk_sb[k][:], wk_e[ts(k, P), :])
        for k in range(NK):
            # zero_bias: host passes Wq TRANSPOSED here (see kernel())
            nc.sync.dma_start(wq_sb[k][:], wq_e[ts(k, P), :])
        for k in range(NK):
            nc.sync.dma_start(wv_sb[k][:], wv_e[ts(k, P), :])

        with tc.tile_pool(name="qpsum", bufs=6, space="PSUM") as qpsum:
            # k (transposed layout, stays local)
            for m in range(NK):
                pk = qpsum.tile([P, SL], F32, tag="pq")
                for k in range(NK):
                    nc.tensor.matmul(
                        pk[:],
                        wk_sb[k][:, ts(m, P)],
                        xnT_sb[:, ts(k, SL)],
                        start=(k == 0),
                        stop=(k == NK - 1),
                    )
                nc.vector.tensor_scalar_add(kT_sb[:, ts(m, SL)], pk[:], bk_col[:, m : m + 1])

            if zero_bias:
                # kappa^T[m, i] = sum_n Wq.T[n, m] * kT[n, i]
                for m in range(NK):
                    pq = qpsum.tile([P, SL], F32, tag="pq")
                    for n in range(NK):
                        nc.tensor.matmul(
                            pq[:],
                            wq_sb[n][:, ts(m, P)],
                            kT_sb[:, ts(n, SL)],
                            start=(n == 0),
                            stop=(n == NK - 1),
                        )
                    nc.scalar.activation(rhs_sb[:, ts(m, SL)], pq[:], AF.Copy)
            else:
                # general path: q (transposed), then its all-gather
                for m in range(NK):
                    pq = qpsum.tile([P, SL], F32, tag="pq")
                    for k in range(NK):
                        nc.tensor.matmul(
                            pq[:],
                            wq_sb[k][:, ts(m, P)],
                            xnT_sb[:, ts(k, SL)],
                            start=(k == 0),
                            stop=(k == NK - 1),
                        )
                    nc.scalar.activation(
                        qT_sb[:, ts(m, SL)], pq[:], AF.Identity, bias=bq_col[:, m : m + 1]
                    )
                for m in range(NK):
                    nc.gpsimd.dma_start(agq_in[m], qT_sb[:, ts(m, SL)])
                nc.gpsimd.collective_compute(
                    "AllGather", mybir.AluOpType.bypass, replica_groups=rg,
                    ins=[agq_in[:]], outs=[agq_out[:]],
                )
                nc.vector.tensor_copy(rhs_sb[:], kT_sb[:])

            # v (natural layout) + its all-gather
            for i in range(NI):
                for cb in range(2):
                    pv = qpsum.tile([P, 512], F32, tag="pq")
                    if not zero_bias:
                        nc.tensor.matmul(
                            pv[:], ones_row_b[:], bv_row[:1, ts(cb, 512)],
                            start=True, stop=False,
                        )
                    for k in range(NK):
                        nc.tensor.matmul(
                            pv[:],
                            xnT_sb[:, k * SL + i * P : k * SL + (i + 1) * P],
                            wv_sb[k][:, ts(cb, 512)],
                            start=(zero_bias and k == 0),
                            stop=(k == NK - 1),
                        )
                    nc.vector.tensor_copy(v_sb[:, i * N + cb * 512 : i * N + (cb + 1) * 512], pv[:])
            for i in range(NI):
                nc.gpsimd.dma_start(agv_in[i], v_sb[:, ts(i, N)])
            nc.gpsimd.collective_compute(
                "AllGather", mybir.AluOpType.bypass, replica_groups=rg,
                ins=[agv_in[:]], outs=[agv_out[:]],
            )
        leave(qkv_cm)
        leave(xnT_cm)

        # W1 resident; emitted here so it prefetches during attention
        w1_cm, w1p = enter(tc.tile_pool(name="w1p", bufs=1, side="left"))
        w1_sb = [w1p.tile([P, FF], BF16, tag=f"w1{c}", name=f"w1{c}") for c in range(NK)]
        for c in range(NK):
            nc.sync.dma_start(w1_sb[c][:], w1_e[ts(c, P), :])
        b1_row = w1p.tile([1, FF], BF16)
        nc.sync.dma_start(b1_row[:1, :], b1_e[:].rearrange("(a n) -> a n", a=1))

        # =========== Phase 2: logits (transposed) + exp + running sum ===========
        wT_cm, wTp = enter(tc.tile_pool(name="wTp", bufs=1, side="left"))
        wT_sb = wTp.tile([P, NJ * SL], BF16)
        acc = wTp.tile([P, SL], F32)
        nc.vector.memset(acc[:], 0.0)
        with (
            tc.tile_pool(name="qf", bufs=6) as qfp,
            tc.tile_pool(name="wpsum", bufs=6, space="PSUM") as wpsum,
        ):
            for rank in range(R):
                qf = qfp.tile([P, NK * SL], BF16, tag="qf")
                for n in range(NK):
                    eng = nc.sync if (n + rank) % 2 == 0 else nc.scalar
                    eng.dma_start(qf[:, ts(n, SL)], agq_out[rank * NK + n])
                for sub in range(NI):
                    jc = rank * NI + sub
                    pw = wpsum.tile([P, SL], F32, tag="pw")
                    for n in range(NK):
                        nc.tensor.matmul(
                            pw[:],
                            qf[:, n * SL + sub * P : n * SL + (sub + 1) * P],
                            rhs_sb[:, ts(n, SL)],
                            start=(n == 0),
                            stop=(n == NK - 1),
                        )
                    nc.scalar.activation(
                        wT_sb[:, ts(jc, SL)], pw[:], AF.Exp, scale=float(SCALE), bias=zero_col[:]
                    )
                    nc.vector.tensor_add(acc[:], acc[:], wT_sb[:, ts(jc, SL)])
        leave(kT_cm)

        # =========== Phase 3: hT accumulation over all j ===========
        mid_cm, midp = enter(tc.tile_pool(name="midp", bufs=1, side="right"))
        hT_sb = midp.tile([P, NK * SL], BF16)
        ff1T_sb = midp.tile([P, NF * SL], BF16)
        with (
            tc.tile_pool(name="vstream", bufs=8) as vsp,
            tc.tile_pool(name="hpsum", bufs=1, space="PSUM") as hpsum,
        ):
            ph = [hpsum.tile([P, SL], F32, tag=f"ph{c}", name=f"ph{c}") for c in range(NK)]
            for j in range(NJ):
                vt = vsp.tile([P, N], BF16, tag="vt")
                (nc.sync if j < 8 else nc.gpsimd).dma_start(vt[:], agv_out[j])
                for c in range(NK):
                    nc.tensor.matmul(
                        ph[c][:],
                        vt[:, ts(c, P)],
                        wT_sb[:, ts(j, SL)],
                        start=(j == 0),
                        stop=(j == NJ - 1),
                    )
            for c in range(NK):
                if c % 2 == 0:
                    nc.scalar.activation(hT_sb[:, ts(c, SL)], ph[c][:], AF.Copy)
                else:
                    nc.vector.tensor_copy(hT_sb[:, ts(c, SL)], ph[c][:])
        # sumexp finalize: PE cost is tiny and overlaps the hT evacuations
        with tc.tile_pool(name="spsum", bufs=2, space="PSUM") as spsum:
            ps = spsum.tile([1, SL], F32, tag="ps")
            nc.tensor.matmul(ps[:], ones_col_f[:], acc[:])
            nc.vector.tensor_copy(sum_row_f[:1, :], ps[:1, :])
            if not zero_bias:
                nc.scalar.activation(sum_row_b[:1, :], ps[:1, :], AF.Copy)
            for ic in range(NI):
                pr = spsum.tile([P, 1], F32, tag="pr")
                nc.tensor.matmul(pr[:], sum_row_f[:1, ts(ic, P)], one_f[:1, :])
                nc.vector.reciprocal(recip_col[:, ic : ic + 1], pr[:])
        # deferred residual affine: xn_sb = z*g + b, done during idle DVE time
        with (
            tc.tile_pool(name="bc", bufs=1, side="left") as bc,
            tc.tile_pool(name="bpsum", bufs=2, space="PSUM") as bpsum,
        ):
            ones_row_f = bc.tile([1, P], F32)
            nc.gpsimd.memset(ones_row_f[:], 1.0)
            g_row = bc.tile([1, N], F32)
            nc.gpsimd.dma_start(g_row[:1, :], g_e[:].rearrange("(a n) -> a n", a=1))
            b_row = bc.tile([1, N], F32)
            nc.gpsimd.dma_start(b_row[:1, :], bn_e[:].rearrange("(a n) -> a n", a=1))
            g_bcast = bc.tile([P, N], F32)
            b_bcast = bc.tile([P, N], F32)
            for vec_row, bcast in ((g_row, g_bcast), (b_row, b_bcast)):
                for blk in range(2):
                    pb = bpsum.tile([P, 512], F32, tag="pb")
                    nc.tensor.matmul(pb[:], ones_row_f[:], vec_row[:1, ts(blk, 512)])
                    nc.vector.tensor_copy(bcast[:, ts(blk, 512)], pb[:])
            for i in range(NI):
                xn_i = xn_sb[:, ts(i, N)]
                nc.vector.tensor_mul(xn_i, xn_i, g_bcast[:])
                nc.vector.tensor_add(xn_i, xn_i, b_bcast[:])

        leave(wT_cm)

        # =========== Phase 4: FFN1 (transposed out, leaky via homogeneity) ===========
        with tc.tile_pool(name="fpsum", bufs=6, space="PSUM") as fpsum:
            for f in range(NF):
                pf = fpsum.tile([P, SL], F32, tag="pf")
                if not zero_bias:
                    nc.tensor.matmul(
                        pf[:], b1_row[:1, ts(f, P)], sum_row_b[:1, :],
                        start=True, stop=False,
                    )
                for c in range(NK):
                    nc.tensor.matmul(
                        pf[:],
                        w1_sb[c][:, ts(f, P)],
                        hT_sb[:, ts(c, SL)],
                        start=(zero_bias and c == 0),
                        stop=(c == NK - 1),
                    )
                nc.scalar.activation(ff1T_sb[:, ts(f, SL)], pf[:], AF.Lrelu, alpha=0.1, bias=zero_col[:])
        leave(w1_cm)

        # =========== Phase 5: FFN2 + epilogue (scale, bias, residual) ===========
        with (
            tc.tile_pool(name="ph5", bufs=1) as ph5,
            tc.tile_pool(name="w2s", bufs=8) as w2s,
            tc.tile_pool(name="outp", bufs=8) as outp,
            tc.tile_pool(name="opsum", bufs=1, space="PSUM") as opsum,
        ):
            b2_row = ph5.tile([1, N], BF16)
            nc.sync.dma_start(b2_row[:1, :], b2_e[:].rearrange("(a n) -> a n", a=1))
            po = [
                opsum.tile([P, 512], F32, tag=f"po{i}", name=f"po{i}")
                for i in range(NI * 2)
            ]
            if not zero_bias:
                for ic in range(NI):
                    for mb in range(2):
                        nc.tensor.matmul(
                            po[ic * 2 + mb][:],
                            sum_row_b[:1, ts(ic, P)],
                            b2_row[:1, ts(mb, 512)],
                            start=True, stop=False,
                        )
            # each po skips one late f-column in the main sweep; the skipped
            # column is appended per-po at the end (stop staggering) so the
            # epilogues overlap the final matmuls instead of all waiting for
            # the last one
            w2_last = [None] * NF
            for f in range(NF):
                w2t = w2s.tile([P, N], BF16, tag="w2t", name=f"w2t{f}")
                nc.scalar.dma_start(w2t[:], w2_e[ts(f, P), :])
                if f >= NF - 8:
                    w2_last[f] = w2t
                prev_loaded = None
                for g in range(NI * 2):
                    if f == NF - 8 + g:
                        continue
                    mmi = nc.tensor.matmul(
                        po[g][:],
                        ff1T_sb[:, f * SL + (g // 2) * P : f * SL + (g // 2 + 1) * P],
                        w2t[:, ts(g % 2, 512)],
                        start=(zero_bias and f == 0),
                        stop=False,
                    )
                    # consecutive mb pair shares lhsT: skip the redundant weight load
                    if prev_loaded == g // 2:
                        mmi.ins.ldweights = False
                    prev_loaded = g // 2
            for g in range(NI * 2):
                ic, mb = g // 2, g % 2
                f = NF - 8 + g
                nc.tensor.matmul(
                    po[g][:],
                    ff1T_sb[:, f * SL + ic * P : f * SL + (ic + 1) * P],
                    w2_last[f][:, ts(mb, 512)],
                    start=False,
                    stop=True,
                )
                ot = outp.tile([P, 512], F32, tag="ot")
                if g % 2 == 0:
                    nc.vector.scalar_tensor_tensor(
                        ot[:],
                        po[g][:],
                        recip_col[:, ic : ic + 1],
                        xn_sb[:, ic * N + mb * 512 : ic * N + (mb + 1) * 512],
                        op0=mybir.AluOpType.mult,
                        op1=mybir.AluOpType.add,
                    )
                else:
                    nc.scalar.activation(
                        ot[:], po[g][:], AF.Identity, scale=recip_col[:, ic : ic + 1]
                    )
                    nc.vector.tensor_add(
                        ot[:], ot[:], xn_sb[:, ic * N + mb * 512 : ic * N + (mb + 1) * 512]
                    )
                oeng = (nc.sync, nc.scalar, nc.gpsimd)[g % 3]
                oeng.dma_start(out_e[ts(ic, P), ts(mb, 512)], ot[:])
        leave(mid_cm)
        leave(base_cm)

    nc.compile()
    return nc


def _get_nc(key):
    global _cached
    if _cached is None:
        _cached = {}
    if key not in _cached:
        if key == "fp8":
            _cached[key] = _build_fp8()
        else:
            _cached[key] = _build(key)
    return _cached[key]


def kernel(**inputs):
    zero_bias = all(
        not np.any(np.asarray(inputs[k], dtype=np.float32))
        for k in ("bq", "bk", "bv", "b1", "b2")
    )
    unit_affine = (
        np.all(np.asarray(inputs["norm_g"], dtype=np.float32) == 1.0)
        and not np.any(np.asarray(inputs["norm_b"], dtype=np.float32))
    )
    if zero_bias and unit_affine:
        nc = _get_nc("fp8")
        in_maps = _fp8_in_maps(inputs)
        res = run_bass_kernel_spmd(nc, in_maps, list(range(R)))
        return np.concatenate([res.results[r]["out"] for r in range(R)], axis=0)

    nc = _get_nc(zero_bias)
    bf = lambda a: np.asarray(a, dtype=np.float32).astype(ml_dtypes.bfloat16)
    f = lambda a: np.ascontiguousarray(np.asarray(a, dtype=np.float32))
    x = f(inputs["x"])
    common = {
        "norm_g": f(inputs["norm_g"]),
        "norm_b": f(inputs["norm_b"]),
        "wq": bf(np.ascontiguousarray(np.asarray(inputs["Wq"]).T)) if zero_bias else bf(inputs["Wq"]),
        "bq": f(inputs["bq"]),
        "wk": bf(inputs["Wk"]),
        "bk": f(inputs["bk"]),
        "wv": bf(inputs["Wv"]),
        "bv": bf(inputs["bv"]),
        "w1": bf(inputs["W1"]),
        "b1": bf(inputs["b1"]),
        "w2": bf(inputs["W2"]),
        "b2": bf(inputs["b2"]),
    }
    in_maps = [dict(common, x=np.ascontiguousarray(x[r * SL : (r + 1) * SL])) for r in range(R)]
    res = run_bass_kernel_spmd(nc, in_maps, list(range(R)))
    return np.concatenate([res.results[r]["out"] for r in range(R)], axis=0)


if __name__ == "__main__":
    rng = np.random.default_rng(0)
    demo = {
        "x": rng.standard_normal((S, N), dtype=np.float32),
        "norm_g": np.ones(N, np.float32),
        "norm_b": np.zeros(N, np.float32),
        "Wq": rng.standard_normal((N, N), dtype=np.float32) * SCALE,
        "bq": np.zeros(N, np.float32),
        "Wk": rng.standard_normal((N, N), dtype=np.float32) * SCALE,
        "bk": np.zeros(N, np.float32),
        "Wv": rng.standard_normal((N, N), dtype=np.float32) * SCALE,
        "bv": np.zeros(N, np.float32),
        "W1": rng.standard_normal((N, FF), dtype=np.float32) * SCALE,
        "b1": np.zeros(FF, np.float32),
        "W2": rng.standard_normal((FF, N), dtype=np.float32) * (1.0 / np.sqrt(FF)),
        "b2": np.zeros(N, np.float32),
    }
    out = kernel(**demo)
    print("out", out.shape, out.dtype, np.abs(out).mean())
